# revision 1
# baseline (speedup 1.0000x reference)
"""Trainium2 Bass kernel for nn_DecoderWithPositionLayer (8 NeuronCores).

Sharding: 2 groups x 4 cores; group g owns batch g (256 tokens).
Within a group (rank s = core % 4):
  - FFN mm1 hidden-split 4-way, relu h AllGathered (bf16), mm2 output-split.
  - Attention split by query tokens (64 q/core). The relative-position bias
    is reassociated:  bias[q,k] = sum_f qp[q,f] rp[q,k,f],
                      qp[h,q,f] = sum_d q[h,q,d] pos_w[f, h*64+d]
    (pos_b adds a row-constant to logits -> cancels in softmax; dropped).
    The causal mask rides as an extra contraction row of host-transposed rp.
  - q-side activations are exchanged with AllToAll so every core's q slice
    lands at a fixed DRAM address (no rank-dependent addressing on device);
    k/v sides use AllGather.
Activations stay feature-major [feature, token]; the only transposes are
DMA-xbar bf16 transposes for attention V and softmax W. Matmul operands are
bf16 (host-cast weights) with f32 PSUM accumulation; LN/softmax math f32.
"""

import contextlib
import numpy as np
import ml_dtypes

import concourse.bass as bass
import concourse.bacc as bacc_mod
import concourse.tile as tile
from concourse import mybir
from concourse.bass_utils import run_bass_kernel_spmd

BF16 = ml_dtypes.bfloat16
FP32 = mybir.dt.float32
BF = mybir.dt.bfloat16
FR = mybir.dt.float32r

B, S, D, F, HID, H = 2, 256, 1024, 64, 4096, 16
DIM = D // H
G = 4
TLOC = S // G          # 64
HIDL = HID // G        # 1024
P = 128
EPS = 1e-3
NEG = -1e9
OUTD = [3 * D, D, 2 * D, D]
OUTD_S = [o // G for o in OUTD]    # 768, 256, 512, 256
RG = [[0, 1, 2, 3], [4, 5, 6, 7]]

_CACHE = {}


# ------------------------------------------------------------------ host prep
def _prep_in_maps(inp):
    f32 = np.float32
    qT = np.ascontiguousarray(np.transpose(np.asarray(inp["queries"], f32), (0, 2, 1)))
    vT = np.ascontiguousarray(np.transpose(np.asarray(inp["values"], f32), (0, 2, 1)))
    pos_wT = np.ascontiguousarray(np.asarray(inp["pos_w"], f32).T).astype(BF16)

    rp = np.asarray(inp["relative_positions"], f32)
    rpT = np.transpose(rp, (0, 1, 3, 2))                       # [B,S,F,S]
    mask = np.where(np.arange(S)[None, :] <= np.arange(S)[:, None], 0.0, NEG)
    rpT_ext = np.concatenate(
        [rpT, np.broadcast_to(mask[None, :, None, :], (B, S, 1, S))], axis=2
    ).astype(BF16)                                             # [B,S,F+1,S]

    blocks = []
    for i in range(4):
        p = f"b{i}_"
        g = np.asarray(inp[p + "ln_g"], f32)
        be = np.asarray(inp[p + "ln_b"], f32)
        w1 = np.asarray(inp[p + "w1"], f32)
        b1 = np.asarray(inp[p + "b1"], f32)
        w2 = np.asarray(inp[p + "w2"], f32)
        b2 = np.asarray(inp[p + "b2"], f32)
        blocks.append((g[:, None] * w1, b1 + be @ w1, w2, b2))

    in_maps = []
    for c in range(8):
        g, s = c // G, c % G
        m = {
            "xT": qT[g],
            "vT": vT[g],
            "xTslice": np.ascontiguousarray(qT[g][:, s * TLOC:(s + 1) * TLOC]),
            "pos_wT": pos_wT,
            "rpT_ext": np.ascontiguousarray(rpT_ext[g, s * TLOC:(s + 1) * TLOC]),
            "rankhot": np.ascontiguousarray(
                np.broadcast_to(np.eye(G, dtype=f32)[s], (P, G))),
        }
        for i, (w1f, b1f, w2, b2) in enumerate(blocks):
            w1s = np.ascontiguousarray(w1f[:, s * HIDL:(s + 1) * HIDL])
            m[f"w1_{i}"] = w1s.astype(BF16)
            m[f"w1sum_{i}"] = w1s.sum(axis=0, keepdims=True).astype(BF16)
            m[f"b1_{i}"] = b1f[s * HIDL:(s + 1) * HIDL].astype(f32)
            o = OUTD_S[i]
            m[f"w2_{i}"] = np.ascontiguousarray(w2[:, s * o:(s + 1) * o]).astype(BF16)
            m[f"b2_{i}"] = b2[s * o:(s + 1) * o].astype(f32)
        in_maps.append(m)
    return in_maps


# --------------------------------------------------------------- device build
def _build_nc():
    nc = bacc_mod.Bacc(num_devices=8)
    io = {}
    io["xT"] = nc.declare_dram_parameter("xT", [D, S], FP32, False)
    io["vT"] = nc.declare_dram_parameter("vT", [D, S], FP32, False)
    io["xTslice"] = nc.declare_dram_parameter("xTslice", [D, TLOC], FP32, False)
    io["rankhot"] = nc.declare_dram_parameter("rankhot", [P, G], FP32, False)
    io["pos_wT"] = nc.declare_dram_parameter("pos_wT", [D, F], BF, False)
    io["rpT_ext"] = nc.declare_dram_parameter("rpT_ext", [TLOC, F + 1, S], BF, False)
    for i in range(4):
        io[f"w1_{i}"] = nc.declare_dram_parameter(f"w1_{i}", [D, HIDL], BF, False)
        io[f"w1sum_{i}"] = nc.declare_dram_parameter(f"w1sum_{i}", [1, HIDL], BF, False)
        io[f"b1_{i}"] = nc.declare_dram_parameter(f"b1_{i}", [HIDL], FP32, False)
        io[f"w2_{i}"] = nc.declare_dram_parameter(f"w2_{i}", [HID, OUTD_S[i]], BF, False)
        io[f"b2_{i}"] = nc.declare_dram_parameter(f"b2_{i}", [OUTD_S[i]], FP32, False)
    io["x2T_out"] = nc.declare_dram_parameter("x2T_out", [D, TLOC], FP32, True)
    io["o3T_out"] = nc.declare_dram_parameter("o3T_out", [OUTD_S[3], S], FP32, True)

    with tile.TileContext(nc) as tc:
        _program(nc, tc, io)
    nc.finalize()
    return nc


def _ap(t, offset, pattern):
    tensor = t.tensor if isinstance(t, bass.AP) else t.ap().tensor
    return bass.AP(tensor=tensor, offset=offset, ap=pattern)


def _program(nc, tc, io):
    import os
    KMODE = int(os.environ.get("KMODE", "8"))
    NOTR = bool(int(os.environ.get("NOTR", "0")))
    ctx = contextlib.ExitStack()
    sb = ctx.enter_context(tc.tile_pool(name="sb", bufs=2))
    consts = ctx.enter_context(tc.tile_pool(name="consts", bufs=1))
    psum = ctx.enter_context(tc.tile_pool(name="psum", bufs=2, space="PSUM"))
    dram = ctx.enter_context(tc.tile_pool(name="dram", bufs=1, space="DRAM"))

    sync, vec, act, pe, gps = nc.sync, nc.vector, nc.scalar, nc.tensor, nc.gpsimd
    AF = mybir.ActivationFunctionType
    ALU = mybir.AluOpType

    ones_col = consts.tile([P, 1], BF, tag="ones_col")
    vec.memset(ones_col, 1.0)
    ones_row = consts.tile([1, P], BF, tag="ones_row")
    vec.memset(ones_row, 1.0)

    from concourse.masks import make_identity
    ident = consts.tile([P, P], BF, tag="ident")
    make_identity(nc, ident)

    # pos_w^T head-chunked at partition base 0: [64(d), 16(h), 64(f)]
    poswT = consts.tile([DIM, H, F], BF, tag="poswT")
    sync.dma_start(out=poswT,
                   in_=_ap(io["pos_wT"], 0, [[F, DIM], [DIM * F, H], [1, F]]))

    # ---------------------------------------------------------------- helpers
    def ln_scale(xt, T, tag):
        """xt: list of 8 [128,T] f32 SBUF tiles (feature-major).
        Returns (xsc bf16 tiles, negmur bf16 [1,T])."""
        ps_sum = psum.tile([1, T], FP32, tag="ps_stat", bufs=2)
        ps_sq = psum.tile([1, T], FP32, tag="ps_stat", bufs=2)
        xba = sb.tile([P, 8, T], BF, tag="lnxb", bufs=1)
        sqa = sb.tile([P, 8, T], BF, tag="lnsq", bufs=1)
        for k in range(8):
            act.copy(xba[:, k, :], xt[k])
            pe.matmul(ps_sum, ones_col, xba[:, k, :],
                      start=(k == 0), stop=(k == 7))
        for k in range(8):
            act.square(sqa[:, k, :], xt[k])
            pe.matmul(ps_sq, ones_col, sqa[:, k, :],
                      start=(k == 0), stop=(k == 7))
        mu = sb.tile([1, T], FP32, tag="lnstat", bufs=8)
        act.mul(mu, ps_sum, 1.0 / D)
        m2 = sb.tile([1, T], FP32, tag="lnstat", bufs=8)
        act.mul(m2, ps_sq, 1.0 / D)
        ve = sb.tile([1, T], FP32, tag="lnstat", bufs=8)
        vec.tensor_mul(ve, mu, mu)
        vec.tensor_sub(ve, m2, ve)
        vec.tensor_scalar_add(ve, ve, EPS)
        rinv = sb.tile([1, T], FP32, tag="lnstat", bufs=8)
        vec.reciprocal(rinv, ve)
        r = sb.tile([1, T], FP32, tag="lnstat", bufs=8)
        act.sqrt(r, rinv)
        mr = sb.tile([1, T], FP32, tag="lnstat", bufs=8)
        vec.tensor_mul(mr, mu, r)
        negmur = sb.tile([1, T], BF, tag="negmur", bufs=4)
        act.mul(negmur, mr, -1.0)
        rb = sb.tile([1, T], BF, tag="lnstatb", bufs=2)
        act.copy(rb, r)
        ps_b = psum.tile([P, T], FP32, tag="ps_bc", bufs=1)
        pe.matmul(ps_b, ones_row, rb, start=True, stop=True)
        r_bc = sb.tile([P, T], FP32, tag="r_bc", bufs=2)
        vec.tensor_copy(r_bc, ps_b)
        xsca = sb.tile([P, 8, T], BF, tag=f"xsc_{tag}", bufs=1)
        for k in range(8):
            vec.tensor_mul(xsca[:, k, :], xt[k], r_bc)
        return [xsca[:, k, :] for k in range(8)], (negmur, xsca)

    def load_feature_major(handle, T):
        xf = sb.tile([P, 8, T], FP32, tag="xf32", bufs=1)
        gps.dma_start(out=xf, in_=_ap(handle, 0, [[T, P], [P * T, 8], [1, T]]))
        return [xf[:, k, :] for k in range(8)]

    def load_bias_cols(name, nm):
        t = consts.tile([P, nm], FP32, tag=f"bias_{name}")
        sync.dma_start(out=t, in_=_ap(io[name], 0, [[1, P], [P, nm]]))
        return t

    def ffn(i, mv, negmur, emit):
        """mv: 8 bf16 [128,S] moving tiles; emit(m, psum) consumes mm2 out."""
        w1t = sb.tile([P, 8, HIDL], BF, tag="w1", bufs=1)
        act.dma_start(out=w1t, in_=_ap(io[f"w1_{i}"], 0,
                                       [[HIDL, P], [P * HIDL, 8], [1, HIDL]]))
        w1sum = consts.tile([1, HIDL], BF, tag=f"w1sum{i}")
        sync.dma_start(out=w1sum, in_=io[f"w1sum_{i}"].ap())
        b1 = load_bias_cols(f"b1_{i}", 8)
        cc_in = dram.tile([HIDL, S], BF, tag="cc_h", bufs=2)
        hg = dram.tile([HID, S], BF, tag="hg", bufs=2)
        hall = sb.tile([P, 8, S], BF, tag="h", bufs=2)
        for m in range(8):
            ps = psum.tile([P, S], FP32, tag="ps_mm", bufs=2)
            for k in range(8):
                pe.matmul(ps, w1t[:, k, m * P:(m + 1) * P], mv[k],
                          start=(k == 0), stop=False)
            pe.matmul(ps, w1sum[:, m * P:(m + 1) * P], negmur,
                      start=False, stop=True)
            vec.tensor_scalar(hall[:, m, :], ps, b1[:, m:m + 1], 0.0,
                              op0=ALU.add, op1=ALU.max)
        sync.dma_start(out=_ap(cc_in, 0, [[S, P], [P * S, 8], [1, S]]),
                       in_=hall)
        gps.collective_compute("AllGather", ALU.bypass, replica_groups=RG,
                               ins=[cc_in.opt()], outs=[hg.opt()])
        w2t = sb.tile([P, 32, OUTD_S[i]], BF, tag="w2", bufs=1)
        act.dma_start(out=w2t, in_=_ap(io[f"w2_{i}"], 0,
                                       [[OUTD_S[i], P], [P * OUTD_S[i], 32],
                                        [1, OUTD_S[i]]]))
        hgt = sb.tile([P, 32, S], BF, tag="hrem", bufs=1)
        act.dma_start(out=hgt, in_=_ap(hg, 0, [[S, P], [P * S, 32], [1, S]]))
        for m in range(OUTD_S[i] // P):
            ps = psum.tile([P, S], FP32, tag="ps_mm", bufs=2)
            for k in range(32):
                pe.matmul(ps, w2t[:, k, m * P:(m + 1) * P], hgt[:, k, :],
                          start=(k == 0), stop=(k == 31))
            emit(m, ps)

    def softmax_av(s_src, kv_all, a_pairs, nheads=H, do_av=True):
        """s_src(j) -> scores [128,S] for head pair j. kv_all: SBUF
        [64(d), 16(h), 2(k/v), S]. a_pairs[j] = psum [128(2h*64d), TLOC]."""
        ens = []
        for j in range(nheads // 2):
            s_t = s_src(j)
            e = sb.tile([P, S], BF, tag="e", bufs=2)
            act.activation(e, s_t, AF.Exp)
            z = sb.tile([P, 1], FP32, tag="z", bufs=4)
            vec.reduce_sum(z, e, axis=mybir.AxisListType.X)
            rz = sb.tile([P, 1], FP32, tag="rz", bufs=4)
            vec.reciprocal(rz, z)
            en = sb.tile([P, S], BF, tag="en", bufs=8)
            vec.tensor_scalar_mul(en, e, rz)
            ens.append(en)
        if not do_av:
            return
        for j in range(nheads // 2):
            en = ens[j]
            a_ps = psum.tile([P, TLOC], FP32, tag="ps_attn", bufs=2)
            for hh in range(2):
                h = 2 * j + hh
                hb = slice(hh * 64, (hh + 1) * 64)
                idb = ident[hb, hb]
                for kc in range(2):
                    # wT [128(k), 64(q)] and v [128(k), 64(d)] via PE transpose
                    wt_ps = psum.tile([P, TLOC], BF, tag="ps_tr", bufs=1)
                    pe.matmul(wt_ps, en[hb, kc * P:(kc + 1) * P], idb,
                              is_transpose=True)
                    wt = sb.tile([P, TLOC], BF, tag="wt", bufs=4)
                    act.copy(wt, wt_ps)
                    vt_ps = psum.tile([P, DIM], BF, tag="ps_tr", bufs=1)
                    pe.matmul(vt_ps, kv_all[:, h, 1, kc * P:(kc + 1) * P],
                              ident[0:64, 0:64], is_transpose=True)
                    vt = sb.tile([P, DIM], BF, tag="vt", bufs=4)
                    act.copy(vt, vt_ps)
                    pe.matmul(a_ps[hh * DIM:(hh + 1) * DIM, :], vt, wt,
                              start=(kc == 0), stop=(kc == 1))
            a_pairs.append(a_ps)

    # =================== LN0 + block0 ===================
    xt0 = load_feature_major(io["xT"], S)
    xsc0, (negmur0, _) = ln_scale(xt0, S, "ln0")
    b2_0 = load_bias_cols("b2_0", 6)
    rankhot = consts.tile([P, G], FP32, tag="rankhot")
    sync.dma_start(out=rankhot, in_=io["rankhot"].ap())
    # q rides a ReduceScatter (one-hot-masked replicas -> pure selection, so
    # every core's own q-token slice lands at a fixed address); k/v AllGather.
    # cc_qrs: [shard j(receiver), r'(sender block), (hl,d), t(recv tokens)]
    cc_qrs = dram.tile([G, G, G * DIM, TLOC], BF, tag="cc_qrs")
    qg = dram.tile([D, TLOC], BF, tag="qg")
    cc_kv0 = dram.tile([2 * G * DIM, S], BF, tag="cc_kv0")     # [512,256]
    kvg0 = dram.tile([2 * D, S], BF, tag="kvg0")
    QRS_J = G * G * DIM * TLOC      # shard stride (elems)
    QRS_R = G * DIM * TLOC          # sender-block stride

    def emit0(m, ps):
        o = sb.tile([P, S], BF, tag="qkvband", bufs=4)
        for j in range(2):
            colbase = m * P + j * 64
            hl, part = colbase // 192, (colbase % 192) // 64   # part:0=q 1=k 2=v
            band = slice(j * 64, (j + 1) * 64)
            if part == 0:
                for rp_ in range(G):
                    oq = sb.tile([P, S], BF, tag="qband", bufs=4)
                    vec.tensor_scalar(oq[band, :], ps[band, :],
                                      b2_0[band, m:m + 1], rankhot[band, rp_:rp_ + 1],
                                      op0=ALU.add, op1=ALU.mult)
                    sync.dma_start(
                        out=_ap(cc_qrs, rp_ * QRS_R + hl * DIM * TLOC,
                                [[TLOC, DIM], [QRS_J, G], [1, TLOC]]),
                        in_=oq[band, :])
            else:
                sc = 0.125 if part == 1 else 1.0
                vec.tensor_scalar(o[band, :], ps[band, :],
                                  b2_0[band, m:m + 1], sc, op0=ALU.add, op1=ALU.mult)
                row = hl * 2 * DIM + (part - 1) * DIM
                sync.dma_start(out=cc_kv0[row:row + DIM, :], in_=o[band, :])

    ffn(0, xsc0, negmur0, emit0)
    gps.collective_compute("ReduceScatter", ALU.add, replica_groups=RG,
                           ins=[cc_qrs.opt()], outs=[qg.opt()])
    gps.collective_compute("AllGather", ALU.bypass, replica_groups=RG,
                           ins=[cc_kv0.opt()], outs=[kvg0.opt()])

    if KMODE < 2:
        ctx.close()
        return
    # =================== LN2 + block2 (cross kv) ===================
    xt2 = load_feature_major(io["vT"], S)
    xsc2, (negmur2, _) = ln_scale(xt2, S, "ln2")
    b2_2 = load_bias_cols("b2_2", 4)
    cc_kv2 = dram.tile([2 * G * DIM, S], BF, tag="cc_kv2")
    kvg2 = dram.tile([2 * D, S], BF, tag="kvg2")

    def emit2(m, ps):
        o = sb.tile([P, S], BF, tag="qkvband", bufs=4)
        for j in range(2):
            colbase = m * P + j * 64
            hl, part = colbase // 128, (colbase % 128) // 64   # 0=k 1=v
            sc = 0.125 if part == 0 else 1.0
            band = slice(j * 64, (j + 1) * 64)
            vec.tensor_scalar(o[band, :], ps[band, :],
                              b2_2[band, m:m + 1], sc, op0=ALU.add, op1=ALU.mult)
            row = hl * 2 * DIM + part * DIM
            sync.dma_start(out=cc_kv2[row:row + DIM, :], in_=o[band, :])

    ffn(2, xsc2, negmur2, emit2)
    gps.collective_compute("AllGather", ALU.bypass, replica_groups=RG,
                           ins=[cc_kv2.opt()], outs=[kvg2.opt()])

    if KMODE < 3:
        ctx.close()
        return
    # =================== self-attention ===================
    # qp^T per head -> qp_ext [65(f), 16(h), 64(q)] with ones row for mask
    qh_all = sb.tile([DIM, H, TLOC], BF, tag="qh", bufs=1)
    act.dma_start(out=qh_all, in_=_ap(qg, 0,
                                      [[TLOC, DIM], [DIM * TLOC, H], [1, TLOC]]))
    qht = [qh_all[:, h, :] for h in range(H)]
    kv0_all = sb.tile([DIM, H, 2, S], BF, tag="kva", bufs=1)
    act.dma_start(out=kv0_all, in_=_ap(kvg0, 0,
                                       [[S, DIM], [2 * DIM * S, H],
                                        [DIM * S, 2], [1, S]]))
    qp_ext = sb.tile([F + 1, H, TLOC], BF, tag="qp_ext", bufs=1)
    vec.memset(qp_ext[F:F + 1, :, :], 1.0)
    for h in range(H):
        qp_ps = psum.tile([F, TLOC], FP32, tag="ps_attn", bufs=2)
        pe.matmul(qp_ps, poswT[:, h, :], qht[h], start=True, stop=True)
        vec.tensor_copy(qp_ext[0:F, h, :], qp_ps)
    # bias per q: [16(h), S] = qp_ext[:,:,q].T @ rpT_ext[q]
    bias_d = dram.tile([TLOC, H, S], BF, tag="bias_d")
    for q8 in range(0, TLOC, 8):
        rp8 = sb.tile([F + 1, 8, S], BF, tag="rpt", bufs=2)
        sync.dma_start(out=rp8, in_=_ap(io["rpT_ext"], q8 * (F + 1) * S,
                                        [[S, F + 1], [(F + 1) * S, 8], [1, S]]))
        for q0 in range(q8, q8 + 8, 4):
            bp = psum.tile([P, S], FP32, tag="ps_attn", bufs=2)
            for qi in range(4):
                pe.matmul(bp[qi * 32:qi * 32 + H, :], qp_ext[:, :, q0 + qi],
                          rp8[:, q0 + qi - q8, :],
                          start=True, stop=True, tile_position=(0, qi * 32))
            bsb = sb.tile([P, S], BF, tag="bsb", bufs=4)
            for qi in range(4):
                rows = slice(qi * 32, qi * 32 + H)
                vec.tensor_copy(bsb[rows, :], bp[rows, :])
                gps.dma_start(out=bias_d[q0 + qi, :, :], in_=bsb[rows, :])
    if KMODE < 4:
        ctx.close()
        return
    # scores + bias -> softmax -> AV
    a0_pairs = []

    def s_src0(j):
        s_ps = psum.tile([P, S], FP32, tag="ps_attn", bufs=2)
        for hh in range(2):
            h = 2 * j + hh
            pe.matmul(s_ps[hh * 64:(hh + 1) * 64, :], qht[h], kv0_all[:, h, 0, :],
                      start=True, stop=True)
        bp = sb.tile([P, S], BF, tag="bp", bufs=4)
        gps.dma_start(
            out=bp, in_=_ap(bias_d.tensor, 2 * j * S,
                            [[S, 2], [H * S, TLOC], [1, S]]))
        s_sb = sb.tile([P, S], FP32, tag="s_sb", bufs=2)
        vec.tensor_add(s_sb, s_ps, bp)
        return s_sb

    softmax_av(s_src0, kv0_all, a0_pairs, do_av=(KMODE >= 5))

    if KMODE < 5:
        ctx.close()
        return
    # x1 = queries_slice + a0 ; LN1 -> AllGather (with negmur row)
    xs_all = sb.tile([P, 8, TLOC], FP32, tag="xslice", bufs=1)
    gps.dma_start(out=xs_all, in_=_ap(io["xTslice"], 0,
                                      [[TLOC, P], [P * TLOC, 8], [1, TLOC]]))
    x1a = sb.tile([P, 8, TLOC], FP32, tag="x1", bufs=1)
    x1t = []
    for k in range(8):
        vec.tensor_add(x1a[:, k, :], xs_all[:, k, :], a0_pairs[k])
        x1t.append(x1a[:, k, :])
    xsc1, (negmur1, xsc1a) = ln_scale(x1t, TLOC, "ln1")
    cc_x1 = dram.tile([D + 1, TLOC], BF, tag="cc_x1")
    x1ng = dram.tile([G, D + 1, TLOC], BF, tag="x1ng")
    sync.dma_start(out=_ap(cc_x1, 0, [[TLOC, P], [P * TLOC, 8], [1, TLOC]]),
                   in_=xsc1a)
    sync.dma_start(out=cc_x1[D:D + 1, :], in_=negmur1)
    gps.collective_compute("AllGather", ALU.bypass, replica_groups=RG,
                           ins=[cc_x1.opt()], outs=[x1ng.opt()])

    if KMODE < 6:
        ctx.close()
        return
    # =================== block1 (y = cross-attn queries) ===================
    st1 = (D + 1) * TLOC
    mv1a = sb.tile([P, 8, S], BF, tag="agin", bufs=1)
    for r_ in range(G):
        act.dma_start(out=mv1a[:, :, r_ * TLOC:(r_ + 1) * TLOC],
                      in_=_ap(x1ng.tensor, r_ * st1,
                              [[TLOC, P], [P * TLOC, 8], [1, TLOC]]))
    mv1 = [mv1a[:, k, :] for k in range(8)]
    negmur1f = sb.tile([1, S], BF, tag="negmur", bufs=4)
    sync.dma_start(out=negmur1f, in_=_ap(x1ng.tensor, D * TLOC,
                                         [[1, 1], [st1, G], [1, TLOC]]))
    b2_1 = load_bias_cols("b2_1", 2)
    cc_yrs = dram.tile([G, G, G * DIM, TLOC], BF, tag="cc_yrs")
    yg = dram.tile([D, TLOC], BF, tag="yg")

    def emit1(m, ps):
        for rp_ in range(G):
            o = sb.tile([P, S], BF, tag="yband", bufs=4)
            vec.tensor_scalar(o, ps, b2_1[:, m:m + 1], rankhot[:, rp_:rp_ + 1],
                              op0=ALU.add, op1=ALU.mult)
            sync.dma_start(
                out=_ap(cc_yrs, rp_ * QRS_R + m * P * TLOC,
                        [[TLOC, P], [QRS_J, G], [1, TLOC]]),
                in_=o)

    ffn(1, mv1, negmur1f, emit1)
    gps.collective_compute("ReduceScatter", ALU.add, replica_groups=RG,
                           ins=[cc_yrs.opt()], outs=[yg.opt()])

    if KMODE < 7:
        ctx.close()
        return
    # =================== cross-attention ===================
    a1_pairs = []
    yh_all = sb.tile([DIM, H, TLOC], BF, tag="yh", bufs=1)
    act.dma_start(out=yh_all, in_=_ap(yg, 0,
                                      [[TLOC, DIM], [DIM * TLOC, H], [1, TLOC]]))
    kv2_all = sb.tile([DIM, H, 2, S], BF, tag="kva", bufs=1)
    act.dma_start(out=kv2_all, in_=_ap(kvg2, 0,
                                       [[S, DIM], [2 * DIM * S, H],
                                        [DIM * S, 2], [1, S]]))

    def s_src1(j):
        s_ps = psum.tile([P, S], FP32, tag="ps_attn", bufs=2)
        for hh in range(2):
            h = 2 * j + hh
            pe.matmul(s_ps[hh * 64:(hh + 1) * 64, :], yh_all[:, h, :],
                      kv2_all[:, h, 0, :], start=True, stop=True)
        return s_ps

    softmax_av(s_src1, kv2_all, a1_pairs)

    # x2 = x1 + a1 -> output + LN3 -> AllGather
    x2a = sb.tile([P, 8, TLOC], FP32, tag="x2", bufs=1)
    x2t = []
    for k in range(8):
        vec.tensor_add(x2a[:, k, :], x1t[k], a1_pairs[k])
        x2t.append(x2a[:, k, :])
    gps.dma_start(out=_ap(io["x2T_out"], 0, [[TLOC, P], [P * TLOC, 8], [1, TLOC]]),
                  in_=x2a)
    xsc3, (negmur3, xsc3a) = ln_scale(x2t, TLOC, "ln3")
    cc_x3 = dram.tile([D + 1, TLOC], BF, tag="cc_x3")
    x3ng = dram.tile([G, D + 1, TLOC], BF, tag="x3ng")
    sync.dma_start(out=_ap(cc_x3, 0, [[TLOC, P], [P * TLOC, 8], [1, TLOC]]),
                   in_=xsc3a)
    sync.dma_start(out=cc_x3[D:D + 1, :], in_=negmur3)
    gps.collective_compute("AllGather", ALU.bypass, replica_groups=RG,
                           ins=[cc_x3.opt()], outs=[x3ng.opt()])

    if KMODE < 8:
        ctx.close()
        return
    # =================== block3 ===================
    mv3a = sb.tile([P, 8, S], BF, tag="agin", bufs=1)
    for r_ in range(G):
        act.dma_start(out=mv3a[:, :, r_ * TLOC:(r_ + 1) * TLOC],
                      in_=_ap(x3ng.tensor, r_ * st1,
                              [[TLOC, P], [P * TLOC, 8], [1, TLOC]]))
    mv3 = [mv3a[:, k, :] for k in range(8)]
    negmur3f = sb.tile([1, S], BF, tag="negmur", bufs=4)
    sync.dma_start(out=negmur3f, in_=_ap(x3ng.tensor, D * TLOC,
                                         [[1, 1], [st1, G], [1, TLOC]]))
    b2_3 = load_bias_cols("b2_3", 2)

    def emit3(m, ps):
        o = sb.tile([P, S], FP32, tag="o3", bufs=2)
        vec.tensor_scalar_add(o, ps, b2_3[:, m:m + 1])
        gps.dma_start(out=io["o3T_out"].ap()[m * P:(m + 1) * P, :], in_=o)

    ffn(3, mv3, negmur3f, emit3)
    ctx.close()


# ------------------------------------------------------------------- runner
def kernel(**inputs) -> np.ndarray:
    if "nc" not in _CACHE:
        _CACHE["nc"] = _build_nc()
    nc = _CACHE["nc"]
    in_maps = _prep_in_maps(inputs)
    res = run_bass_kernel_spmd(nc, in_maps, core_ids=list(range(8)))
    out = np.zeros((B, S, D), np.float32)
    for g in range(B):
        x2T = np.concatenate(
            [np.asarray(res.results[g * G + s]["x2T_out"]) for s in range(G)], axis=1)
        o3T = np.concatenate(
            [np.asarray(res.results[g * G + s]["o3T_out"]) for s in range(G)], axis=0)
        out[g] = (x2T + o3T).T
    return out



# revision 10
# speedup vs baseline: 1.1534x; 1.1534x over previous
"""Trainium2 Bass kernel for nn_DecoderWithPositionLayer (8 NeuronCores).

Sharding: 2 groups x 4 cores; group g owns batch g (256 tokens).
Within a group (rank s = core % 4):
  - Every FFN: mm1 hidden-split 4-way (local h = relu(w1_s^T xln + b1_s)),
    mm2 row-parallel over the local hidden slice producing PARTIAL outputs
    for the FULL output dim; partials are combined with a single collective
    AFTER mm2 (no mid-FFN AllGather):
      * block0 q-part / block1 y: token-reordered ReduceScatter(add)
        -> each core gets its 64 query tokens, full feature dim.
      * block0 kv / block2 kv: AllReduce(add) -> full kv on every core.
      * block3: no collective; f32 partials returned, host sums 4 ranks.
  - Attention split by query tokens (64 q/core). Relative-position bias
    reassociated: bias[q,k] = sum_f qp[q,f] rp[q,k,f],
                  qp[h,q,f] = sum_d q[h,q,d] pos_w[f, h*64+d]
    (pos_b is row-constant in softmax -> dropped). Causal mask rides as an
    extra contraction row of host-transposed rp.
  - Softmax-weight and attention-V transposes use the DMA xbar (bf16).
  - b2 biases are folded into partials as 0.25*b2 per rank (sums to b2).
Activations stay feature-major [feature, token]; matmuls bf16 with f32
PSUM accumulation; LN/softmax math f32.
"""

import contextlib
import numpy as np
import ml_dtypes

import concourse.bass as bass
import concourse.bacc as bacc_mod
import concourse.tile as tile
from concourse import mybir
from concourse.bass_utils import run_bass_kernel_spmd

BF16 = ml_dtypes.bfloat16
FP32 = mybir.dt.float32
BF = mybir.dt.bfloat16

B, S, D, F, HID, H = 2, 256, 1024, 64, 4096, 16
DIM = D // H
G = 4
TLOC = S // G          # 64
HIDL = HID // G        # 1024
P = 128
EPS = 1e-3
NEG = -1e9
OUTD = [3 * D, D, 2 * D, D]
RG = [[0, 1, 2, 3], [4, 5, 6, 7]]

_CACHE = {}


# ------------------------------------------------------------------ host prep
def _prep_in_maps(inp):
    f32 = np.float32
    qT = np.ascontiguousarray(np.transpose(np.asarray(inp["queries"], f32), (0, 2, 1)))
    vT = np.ascontiguousarray(np.transpose(np.asarray(inp["values"], f32), (0, 2, 1)))
    pos_wT = np.ascontiguousarray(np.asarray(inp["pos_w"], f32).T).astype(BF16)

    rp = np.asarray(inp["relative_positions"], f32)
    rpT = np.transpose(rp, (0, 1, 3, 2))                       # [B,S,F,S]
    mask = np.where(np.arange(S)[None, :] <= np.arange(S)[:, None], 0.0, NEG)
    rpT_ext = np.concatenate(
        [rpT, np.broadcast_to(mask[None, :, None, :], (B, S, 1, S))], axis=2
    ).astype(BF16)                                             # [B,S,F+1,S]

    blocks = []
    for i in range(4):
        p = f"b{i}_"
        g = np.asarray(inp[p + "ln_g"], f32)
        be = np.asarray(inp[p + "ln_b"], f32)
        w1 = np.asarray(inp[p + "w1"], f32)
        b1 = np.asarray(inp[p + "b1"], f32)
        w2 = np.asarray(inp[p + "w2"], f32)
        b2 = np.asarray(inp[p + "b2"], f32)
        blocks.append((g[:, None] * w1, b1 + be @ w1, w2, b2))

    in_maps = []
    for c in range(8):
        g, s = c // G, c % G
        # rp for this core's 64 q tokens, partition-major [F+1, 64, S]
        rp_c = np.ascontiguousarray(
            rpT_ext[g, s * TLOC:(s + 1) * TLOC].transpose(1, 0, 2))
        m = {
            "xT": qT[g],
            "vT": vT[g],
            "xTslice": np.ascontiguousarray(qT[g][:, s * TLOC:(s + 1) * TLOC]),
            "pos_wT": pos_wT,
            "rpT": rp_c,
        }
        for i, (w1f, b1f, w2, b2) in enumerate(blocks):
            w1s = np.ascontiguousarray(w1f[:, s * HIDL:(s + 1) * HIDL])
            m[f"w1_{i}"] = w1s.astype(BF16)
            m[f"w1sum_{i}"] = w1s.sum(axis=0, keepdims=True).astype(BF16)
            m[f"b1_{i}"] = b1f[s * HIDL:(s + 1) * HIDL].astype(f32)
            m[f"w2_{i}"] = np.ascontiguousarray(
                w2[s * HIDL:(s + 1) * HIDL, :]).astype(BF16)
            m[f"b2_{i}"] = (0.25 * b2).astype(f32)
        in_maps.append(m)
    return in_maps


# --------------------------------------------------------------- device build
def _build_nc():
    nc = bacc_mod.Bacc(num_devices=8)
    io = {}
    io["xT"] = nc.declare_dram_parameter("xT", [D, S], FP32, False)
    io["vT"] = nc.declare_dram_parameter("vT", [D, S], FP32, False)
    io["xTslice"] = nc.declare_dram_parameter("xTslice", [D, TLOC], FP32, False)
    io["pos_wT"] = nc.declare_dram_parameter("pos_wT", [D, F], BF, False)
    io["rpT"] = nc.declare_dram_parameter("rpT", [F + 1, TLOC, S], BF, False)
    for i in range(4):
        io[f"w1_{i}"] = nc.declare_dram_parameter(f"w1_{i}", [D, HIDL], BF, False)
        io[f"w1sum_{i}"] = nc.declare_dram_parameter(f"w1sum_{i}", [1, HIDL], BF, False)
        io[f"b1_{i}"] = nc.declare_dram_parameter(f"b1_{i}", [HIDL], FP32, False)
        io[f"w2_{i}"] = nc.declare_dram_parameter(f"w2_{i}", [HIDL, OUTD[i]], BF, False)
        io[f"b2_{i}"] = nc.declare_dram_parameter(f"b2_{i}", [OUTD[i]], FP32, False)
    io["x2T_out"] = nc.declare_dram_parameter("x2T_out", [D, TLOC], FP32, True)
    io["o3T_out"] = nc.declare_dram_parameter("o3T_out", [D, S], FP32, True)

    with tile.TileContext(nc) as tc:
        _program(nc, tc, io)
    nc.finalize()
    return nc


def _ap(t, offset, pattern):
    tensor = t.tensor if isinstance(t, bass.AP) else t.ap().tensor
    return bass.AP(tensor=tensor, offset=offset, ap=pattern)


def _program(nc, tc, io):
    import os
    KMODE = int(os.environ.get("KMODE", "9"))
    ctx = contextlib.ExitStack()
    sb = ctx.enter_context(tc.tile_pool(name="sb", bufs=2))
    consts = ctx.enter_context(tc.tile_pool(name="consts", bufs=1))
    psum = ctx.enter_context(tc.tile_pool(name="psum", bufs=2, space="PSUM"))
    dram = ctx.enter_context(tc.tile_pool(name="dram", bufs=1, space="DRAM"))

    sync, vec, act, pe, gps = nc.sync, nc.vector, nc.scalar, nc.tensor, nc.gpsimd
    AF = mybir.ActivationFunctionType
    ALU = mybir.AluOpType

    ones_col = consts.tile([P, 1], BF, tag="ones_col")
    vec.memset(ones_col, 1.0)
    ones_row = consts.tile([1, P], BF, tag="ones_row")
    vec.memset(ones_row, 1.0)

    # ---- t0 prefetches ----
    # pos_w^T head-chunked: [64(d), 16(h), 64(f)]
    poswT = consts.tile([DIM, H, F], BF, tag="poswT")
    sync.dma_start(out=poswT,
                   in_=_ap(io["pos_wT"], 0, [[F, DIM], [DIM * F, H], [1, F]]))
    # queries slice for the x1 residual (own 64 tokens)
    xs_all = consts.tile([P, 8, TLOC], FP32, tag="xslice")
    sync.dma_start(out=xs_all, in_=_ap(io["xTslice"], 0,
                                       [[TLOC, P], [P * TLOC, 8], [1, TLOC]]))

    # ---------------------------------------------------------------- helpers
    def ln_scale(xt, T, tag):
        """xt: list of 8 [128,T] f32 SBUF tiles (feature-major).
        Returns (xsc bf16 tiles, negmur bf16 [1,T], xsca full tile)."""
        ps_sum = psum.tile([1, T], FP32, tag="ps_stat", bufs=2)
        ps_sq = psum.tile([1, T], FP32, tag="ps_stat", bufs=2)
        xba = sb.tile([P, 8, T], BF, tag="lnxb", bufs=1)
        sqa = sb.tile([P, 8, T], BF, tag="lnsq", bufs=1)
        for k in range(8):
            act.copy(xba[:, k, :], xt[k])
            pe.matmul(ps_sum, ones_col, xba[:, k, :],
                      start=(k == 0), stop=(k == 7))
        for k in range(8):
            act.square(sqa[:, k, :], xt[k])
            pe.matmul(ps_sq, ones_col, sqa[:, k, :],
                      start=(k == 0), stop=(k == 7))
        mu = sb.tile([1, T], FP32, tag="lnstat", bufs=8)
        act.mul(mu, ps_sum, 1.0 / D)
        m2 = sb.tile([1, T], FP32, tag="lnstat", bufs=8)
        act.mul(m2, ps_sq, 1.0 / D)
        ve = sb.tile([1, T], FP32, tag="lnstat", bufs=8)
        vec.tensor_mul(ve, mu, mu)
        vec.tensor_sub(ve, m2, ve)
        vec.tensor_scalar_add(ve, ve, EPS)
        rinv = sb.tile([1, T], FP32, tag="lnstat", bufs=8)
        vec.reciprocal(rinv, ve)
        r = sb.tile([1, T], FP32, tag="lnstat", bufs=8)
        act.sqrt(r, rinv)
        mr = sb.tile([1, T], FP32, tag="lnstat", bufs=8)
        vec.tensor_mul(mr, mu, r)
        negmur = sb.tile([1, T], BF, tag="negmur", bufs=4)
        act.mul(negmur, mr, -1.0)
        rb = sb.tile([1, T], BF, tag="lnstatb", bufs=2)
        act.copy(rb, r)
        ps_b = psum.tile([P, T], FP32, tag="ps_bc", bufs=1)
        pe.matmul(ps_b, ones_row, rb, start=True, stop=True)
        r_bc = sb.tile([P, T], FP32, tag="r_bc", bufs=2)
        vec.tensor_copy(r_bc, ps_b)
        xsca = sb.tile([P, 8, T], BF, tag=f"xsc_{tag}", bufs=1)
        for k in range(8):
            vec.tensor_mul(xsca[:, k, :], xt[k], r_bc)
        return [xsca[:, k, :] for k in range(8)], negmur, xsca

    def load_feature_major(handle, T, eng=sync):
        xf = sb.tile([P, 8, T], FP32, tag="xf32", bufs=1)
        eng.dma_start(out=xf, in_=_ap(handle, 0, [[T, P], [P * T, 8], [1, T]]))
        return [xf[:, k, :] for k in range(8)]

    def load_bias_cols(name, nm, eng=sync):
        t = consts.tile([P, nm], FP32, tag=f"bias_{name}")
        eng.dma_start(out=t, in_=_ap(io[name], 0, [[1, P], [P, nm]]))
        return t

    def ffn(i, mv, negmur, emit):
        """mm1 (hidden-split) -> relu -> mm2 row-parallel over local hidden.
        emit(m, ps) consumes the f32 partial [128, S] for out rows m*128.."""
        w1t = sb.tile([P, 8, HIDL], BF, tag="w1", bufs=2)
        act.dma_start(out=w1t, in_=_ap(io[f"w1_{i}"], 0,
                                       [[HIDL, P], [P * HIDL, 8], [1, HIDL]]))
        w1sum = consts.tile([1, HIDL], BF, tag=f"w1sum{i}")
        sync.dma_start(out=w1sum, in_=io[f"w1sum_{i}"].ap())
        b1 = load_bias_cols(f"b1_{i}", 8)
        od = OUTD[i]
        w2r = sb.tile([P, 8, od], BF, tag="w2", bufs=1,
                      padded_shape=[P, 8, 3 * D])
        act.dma_start(out=w2r, in_=_ap(io[f"w2_{i}"], 0,
                                       [[od, P], [P * od, 8], [1, od]]))
        h = sb.tile([P, 8, S], BF, tag="h", bufs=2)
        for m in range(8):
            ps = psum.tile([P, S], FP32, tag="ps_mm", bufs=2)
            for k in range(8):
                pe.matmul(ps, w1t[:, k, m * P:(m + 1) * P], mv[k],
                          start=(k == 0), stop=False)
            pe.matmul(ps, w1sum[:, m * P:(m + 1) * P], negmur,
                      start=False, stop=True)
            vec.tensor_scalar(h[:, m, :], ps, b1[:, m:m + 1], 0.0,
                              op0=ALU.add, op1=ALU.max)
        for m in range(od // P):
            ps = psum.tile([P, S], FP32, tag="ps_mm", bufs=2)
            for k in range(8):
                pe.matmul(ps, w2r[:, k, m * P:(m + 1) * P], h[:, k, :],
                          start=(k == 0), stop=(k == 7))
            emit(m, ps)

    # =================== LN0 + block0 (fused qkv) ===================
    xt0 = load_feature_major(io["xT"], S)
    xsc0, negmur0, _ = ln_scale(xt0, S, "ln0")
    b2_0 = load_bias_cols("b2_0", 24)
    # token-reordered partial q -> ReduceScatter; kv partials -> AllReduce
    cc_q0 = dram.tile([G, D, TLOC], BF, tag="cc_q0")
    qg = dram.tile([D, TLOC], BF, tag="qg")
    cc_kv0 = dram.tile([2 * D, S], BF, tag="cc_kv0")
    kvg0 = dram.tile([2 * D, S], BF, tag="kvg0")

    def emit0(m, ps):
        o = sb.tile([P, S], BF, tag="qkvband", bufs=4)
        for j in range(2):
            colbase = m * P + j * 64
            hd, part = colbase // 192, (colbase % 192) // 64   # 0=q 1=k 2=v
            band = slice(j * 64, (j + 1) * 64)
            if part == 0:
                vec.tensor_scalar_add(o[band, :], ps[band, :], b2_0[band, m:m + 1])
                sync.dma_start(
                    out=_ap(cc_q0, hd * DIM * TLOC,
                            [[TLOC, DIM], [D * TLOC, G], [1, TLOC]]),
                    in_=o[band, :])
            else:
                sc = 0.125 if part == 1 else 1.0
                vec.tensor_scalar(o[band, :], ps[band, :],
                                  b2_0[band, m:m + 1], sc, op0=ALU.add, op1=ALU.mult)
                row = hd * 2 * DIM + (part - 1) * DIM
                act.dma_start(out=cc_kv0[row:row + DIM, :], in_=o[band, :])

    ffn(0, xsc0, negmur0, emit0)
    gps.collective_compute("ReduceScatter", ALU.add, replica_groups=RG,
                           ins=[cc_q0.opt()], outs=[qg.opt()])
    gps.collective_compute("AllReduce", ALU.add, replica_groups=RG,
                           ins=[cc_kv0.opt()], outs=[kvg0.opt()])

    if KMODE < 2:
        ctx.close()
        return

    # =================== LN2 + block2 (cross kv) ===================
    xt2 = load_feature_major(io["vT"], S)
    xsc2, negmur2, _ = ln_scale(xt2, S, "ln2")
    b2_2 = load_bias_cols("b2_2", 16)
    cc_kv2 = dram.tile([2 * D, S], BF, tag="cc_kv2")
    kvg2 = dram.tile([2 * D, S], BF, tag="kvg2")

    def emit2(m, ps):
        o = sb.tile([P, S], BF, tag="qkvband", bufs=4)
        for j in range(2):
            colbase = m * P + j * 64
            hd, part = colbase // 128, (colbase % 128) // 64   # 0=k 1=v
            sc = 0.125 if part == 0 else 1.0
            band = slice(j * 64, (j + 1) * 64)
            vec.tensor_scalar(o[band, :], ps[band, :],
                              b2_2[band, m:m + 1], sc, op0=ALU.add, op1=ALU.mult)
            row = hd * 2 * DIM + part * DIM
            act.dma_start(out=cc_kv2[row:row + DIM, :], in_=o[band, :])

    ffn(2, xsc2, negmur2, emit2)
    gps.collective_compute("AllReduce", ALU.add, replica_groups=RG,
                           ins=[cc_kv2.opt()], outs=[kvg2.opt()])

    if KMODE < 3:
        ctx.close()
        return

    # =================== qp + relative-position bias ===================
    # qh: [64(d), 16(h), 64(q)]
    qh_all = sb.tile([DIM, H, TLOC], BF, tag="qh", bufs=1)
    sync.dma_start(out=qh_all, in_=_ap(qg, 0,
                                       [[TLOC, DIM], [DIM * TLOC, H], [1, TLOC]]))
    qp_ext = sb.tile([F + 1, H, TLOC], BF, tag="qp_ext", bufs=1)
    vec.memset(qp_ext[F:F + 1, :, :], 1.0)
    for h in range(H):
        qp_ps = psum.tile([F, TLOC], FP32, tag="ps_attn", bufs=2)
        pe.matmul(qp_ps, poswT[:, h, :], qh_all[:, h, :], start=True, stop=True)
        vec.tensor_copy(qp_ext[0:F, h, :], qp_ps)
    # bias per q-group of 4: bp rows qi*32+h, cols k; batched DRAM round trip.
    # rp streamed per q-octet: [65(f), 8(q), 256(k)]
    bias_d = dram.tile([TLOC // 4, P, S], BF, tag="bias_d")
    for oc in range(TLOC // 8):
        rp8 = sb.tile([F + 1, 8, S], BF, tag="rp8", bufs=2)
        sync.dma_start(out=rp8, in_=_ap(io["rpT"], oc * 8 * S,
                                        [[TLOC * S, F + 1], [S, 8], [1, S]]))
        for g2 in range(2):
            g = oc * 2 + g2
            bp = psum.tile([P, S], FP32, tag="ps_attn", bufs=2)
            for qi in range(4):
                q = g * 4 + qi
                pe.matmul(bp[qi * 32:qi * 32 + H, :], qp_ext[:, :, q],
                          rp8[:, q - oc * 8, :], start=True, stop=True,
                          tile_position=(0, qi * 32))
            bsb = sb.tile([P, S], BF, tag="bsb", bufs=2)
            vec.tensor_copy(bsb, bp)
            act.dma_start(out=bias_d[g], in_=bsb)

    if KMODE < 4:
        ctx.close()
        return

    # =================== attention machinery ===================
    def load_k(kvg, tag):
        k_sb = sb.tile([DIM, H, S], BF, tag=tag, bufs=1)
        sync.dma_start(out=k_sb, in_=_ap(kvg, 0,
                                         [[S, DIM], [2 * DIM * S, H], [1, S]]))
        return k_sb

    def load_vT(kvg, tag):
        # vT: [128(k-token), 16(h), 2(kc), 64(d)] via DMA-xbar transposes
        vt = sb.tile([P, H, 2, DIM], BF, tag=tag, bufs=1)
        for h in range(H):
            row = h * 2 * DIM + DIM
            for kc in range(2):
                eng = act if (h % 2) else sync
                eng.dma_start(out=vt[:, h, kc, :],
                              in_=kvg[row:row + DIM, kc * P:(kc + 1) * P],
                              transpose=True)
        return vt

    def softmax_av(s_src, vt, consume):
        """s_src(j) -> f32 scores-ish [128,S] (psum or sbuf) for head pair j.
        vt: [128, H, 2, DIM]. consume(j, a_ps) eats psum [128(2h*64d), TLOC]."""
        for j in range(H // 2):
            s_t = s_src(j)
            e = sb.tile([P, S], BF, tag="e", bufs=2)
            act.activation(e, s_t, AF.Exp)
            z = sb.tile([P, 1], FP32, tag="z", bufs=4)
            vec.reduce_sum(z, e, axis=mybir.AxisListType.X)
            rz = sb.tile([P, 1], FP32, tag="rz", bufs=4)
            vec.reciprocal(rz, z)
            en = sb.tile([P, S], BF, tag="en", bufs=2)
            vec.tensor_scalar_mul(en, e, rz)
            wt = sb.tile([P, 2, P], BF, tag="wt", bufs=2)
            for kc in range(2):
                eng = act if kc else sync
                eng.dma_start(out=wt[:, kc, :], in_=en[:, kc * P:(kc + 1) * P],
                              transpose=True)
            a_ps = psum.tile([P, TLOC], FP32, tag="ps_av", bufs=1)
            for kc in range(2):
                for hh in range(2):
                    pe.matmul(a_ps[hh * DIM:(hh + 1) * DIM, :],
                              vt[:, 2 * j + hh, kc, :], wt[:, kc, hh * 64:(hh + 1) * 64],
                              start=(kc == 0), stop=(kc == 1))
            consume(j, a_ps)

    # =================== self-attention ===================
    k0_sb = load_k(kvg0, "k0")
    vt0 = load_vT(kvg0, "vt0")

    def s_src0(j):
        s_ps = psum.tile([P, S], FP32, tag="ps_attn", bufs=2)
        for hh in range(2):
            h = 2 * j + hh
            pe.matmul(s_ps[hh * 64:(hh + 1) * 64, :], qh_all[:, h, :],
                      k0_sb[:, h, :], start=True, stop=True)
        bp = sb.tile([P, S], BF, tag="bp", bufs=2)
        sync.dma_start(
            out=bp, in_=_ap(bias_d.tensor, 2 * j * S,
                            [[S, 2], [P * S, TLOC // 4], [32 * S, 4], [1, S]]))
        s_sb = sb.tile([P, S], FP32, tag="s_sb", bufs=2)
        vec.tensor_add(s_sb, s_ps, bp)
        return s_sb

    # x1 = queries_slice + a0, built pair by pair
    x1a = sb.tile([P, 8, TLOC], FP32, tag="x1", bufs=1)

    def consume0(j, a_ps):
        vec.tensor_add(x1a[:, j, :], xs_all[:, j, :], a_ps)

    softmax_av(s_src0, vt0, consume0)

    if KMODE < 5:
        ctx.close()
        return

    # LN1 -> AllGather (with negmur row)
    x1t = [x1a[:, k, :] for k in range(8)]
    xsc1, negmur1, xsc1a = ln_scale(x1t, TLOC, "ln1")
    cc_x1 = dram.tile([D + 1, TLOC], BF, tag="cc_x1")
    x1ng = dram.tile([G, D + 1, TLOC], BF, tag="x1ng")
    sync.dma_start(out=_ap(cc_x1, 0, [[TLOC, P], [P * TLOC, 8], [1, TLOC]]),
                   in_=xsc1a)
    sync.dma_start(out=cc_x1[D:D + 1, :], in_=negmur1)
    gps.collective_compute("AllGather", ALU.bypass, replica_groups=RG,
                           ins=[cc_x1.opt()], outs=[x1ng.opt()])

    if KMODE < 6:
        ctx.close()
        return

    # =================== block1 (y = cross-attn queries) ===================
    st1 = (D + 1) * TLOC
    mv1a = sb.tile([P, 8, S], BF, tag="agin", bufs=1)
    for r_ in range(G):
        act.dma_start(out=mv1a[:, :, r_ * TLOC:(r_ + 1) * TLOC],
                      in_=_ap(x1ng.tensor, r_ * st1,
                              [[TLOC, P], [P * TLOC, 8], [1, TLOC]]))
    mv1 = [mv1a[:, k, :] for k in range(8)]
    negmur1f = sb.tile([1, S], BF, tag="negmur", bufs=4)
    sync.dma_start(out=negmur1f, in_=_ap(x1ng.tensor, D * TLOC,
                                         [[1, 1], [st1, G], [1, TLOC]]))
    b2_1 = load_bias_cols("b2_1", 8)
    cc_y = dram.tile([G, D, TLOC], BF, tag="cc_y")
    yg = dram.tile([D, TLOC], BF, tag="yg")

    def emit1(m, ps):
        o = sb.tile([P, S], BF, tag="yband", bufs=4)
        vec.tensor_scalar_add(o, ps, b2_1[:, m:m + 1])
        sync.dma_start(
            out=_ap(cc_y, m * P * TLOC, [[TLOC, P], [D * TLOC, G], [1, TLOC]]),
            in_=o)

    ffn(1, mv1, negmur1f, emit1)
    gps.collective_compute("ReduceScatter", ALU.add, replica_groups=RG,
                           ins=[cc_y.opt()], outs=[yg.opt()])

    if KMODE < 7:
        ctx.close()
        return

    # =================== cross-attention ===================
    k2_sb = load_k(kvg2, "k2")
    vt2 = load_vT(kvg2, "vt2")
    yh_all = sb.tile([DIM, H, TLOC], BF, tag="yh", bufs=1)
    sync.dma_start(out=yh_all, in_=_ap(yg, 0,
                                       [[TLOC, DIM], [DIM * TLOC, H], [1, TLOC]]))

    def s_src1(j):
        s_ps = psum.tile([P, S], FP32, tag="ps_attn", bufs=2)
        for hh in range(2):
            h = 2 * j + hh
            pe.matmul(s_ps[hh * 64:(hh + 1) * 64, :], yh_all[:, h, :],
                      k2_sb[:, h, :], start=True, stop=True)
        return s_ps

    # x2 = x1 + a1 -> output + LN3 -> AllGather
    x2a = sb.tile([P, 8, TLOC], FP32, tag="x2", bufs=1)

    def consume1(j, a_ps):
        vec.tensor_add(x2a[:, j, :], x1a[:, j, :], a_ps)

    softmax_av(s_src1, vt2, consume1)

    x2t = [x2a[:, k, :] for k in range(8)]
    sync.dma_start(out=_ap(io["x2T_out"], 0, [[TLOC, P], [P * TLOC, 8], [1, TLOC]]),
                   in_=x2a)
    xsc3, negmur3, xsc3a = ln_scale(x2t, TLOC, "ln3")
    cc_x3 = dram.tile([D + 1, TLOC], BF, tag="cc_x3")
    x3ng = dram.tile([G, D + 1, TLOC], BF, tag="x3ng")
    sync.dma_start(out=_ap(cc_x3, 0, [[TLOC, P], [P * TLOC, 8], [1, TLOC]]),
                   in_=xsc3a)
    sync.dma_start(out=cc_x3[D:D + 1, :], in_=negmur3)
    gps.collective_compute("AllGather", ALU.bypass, replica_groups=RG,
                           ins=[cc_x3.opt()], outs=[x3ng.opt()])

    if KMODE < 8:
        ctx.close()
        return

    # =================== block3 (partials; host sums ranks) ===================
    mv3a = sb.tile([P, 8, S], BF, tag="agin", bufs=1)
    for r_ in range(G):
        act.dma_start(out=mv3a[:, :, r_ * TLOC:(r_ + 1) * TLOC],
                      in_=_ap(x3ng.tensor, r_ * st1,
                              [[TLOC, P], [P * TLOC, 8], [1, TLOC]]))
    mv3 = [mv3a[:, k, :] for k in range(8)]
    negmur3f = sb.tile([1, S], BF, tag="negmur", bufs=4)
    sync.dma_start(out=negmur3f, in_=_ap(x3ng.tensor, D * TLOC,
                                         [[1, 1], [st1, G], [1, TLOC]]))
    b2_3 = load_bias_cols("b2_3", 8)

    def emit3(m, ps):
        o = sb.tile([P, S], FP32, tag="o3", bufs=1)
        vec.tensor_scalar_add(o, ps, b2_3[:, m:m + 1])
        act.dma_start(out=io["o3T_out"].ap()[m * P:(m + 1) * P, :], in_=o)

    ffn(3, mv3, negmur3f, emit3)
    ctx.close()


# ------------------------------------------------------------------- runner
def kernel(**inputs) -> np.ndarray:
    if "nc" not in _CACHE:
        _CACHE["nc"] = _build_nc()
    nc = _CACHE["nc"]
    in_maps = _prep_in_maps(inputs)
    res = run_bass_kernel_spmd(nc, in_maps, core_ids=list(range(8)))
    out = np.zeros((B, S, D), np.float32)
    for g in range(B):
        x2T = np.concatenate(
            [np.asarray(res.results[g * G + s]["x2T_out"]) for s in range(G)], axis=1)
        o3T = np.sum(
            [np.asarray(res.results[g * G + s]["o3T_out"]) for s in range(G)], axis=0)
        out[g] = (x2T + o3T).T
    return out


# revision 15
# speedup vs baseline: 1.4079x; 1.2206x over previous
"""Trainium2 Bass kernel for nn_DecoderWithPositionLayer (8 NeuronCores).

Sharding: 2 groups x 4 cores; group g owns batch g (256 tokens).
Within a group (rank s = core % 4):
  - Every FFN: mm1 hidden-split 4-way (local h = relu(w1_s^T xln + b1_s)),
    mm2 row-parallel over the local hidden slice producing PARTIAL outputs
    for the FULL output dim; partials are combined with a single collective
    AFTER mm2 (no mid-FFN AllGather):
      * block0 q-part / block1 y: token-reordered ReduceScatter(add)
        -> each core gets its 64 query tokens, full feature dim.
      * block0 kv / block2 kv: AllReduce(add) -> full kv on every core.
      * block3: no collective; f32 partials returned, host sums 4 ranks.
  - b1/b2 bias adds ride as rank-1 PE matmuls into the mm PSUM accumulation
    (b2 pre-scaled 0.25 per rank so the collective sum restores it); the
    1/sqrt(dim) score scale is folded into w2/b2 k-columns on the host.
  - Attention split by query tokens (64 q/core). Relative-position bias
    reassociated: bias[q,k] = sum_f qp[q,f] rp[q,k,f],
                  qp[h,q,f] = sum_d q[h,q,d] pos_w[f, h*64+d]
    (pos_b is row-constant in softmax -> dropped). Causal mask rides as an
    extra contraction row of host-transposed rp. Softmax-weight and V
    transposes are PE is_transpose matmuls.
  - All host-side tensors are laid out partition-major so every big DMA
    moves multi-KB contiguous runs per partition.
Activations stay feature-major [feature, token]; matmuls bf16 with f32
PSUM accumulation; LN/softmax math f32.
"""

import contextlib
import numpy as np
import ml_dtypes

import concourse.bass as bass
import concourse.bacc as bacc_mod
import concourse.tile as tile
from concourse import mybir
from concourse.bass_utils import run_bass_kernel_spmd

BF16 = ml_dtypes.bfloat16
FP32 = mybir.dt.float32
BF = mybir.dt.bfloat16

B, S, D, F, HID, H = 2, 256, 1024, 64, 4096, 16
DIM = D // H
G = 4
TLOC = S // G          # 64
HIDL = HID // G        # 1024
P = 128
EPS = 1e-3
NEG = -1e9
OUTD = [3 * D, D, 2 * D, D]
RG = [[0, 1, 2, 3], [4, 5, 6, 7]]

_CACHE = {}


def _pmajor(a):
    """[128*k, N] -> [128, k, N] partition-major contiguous."""
    rows, n = a.shape
    k = rows // P
    return np.ascontiguousarray(a.reshape(k, P, n).transpose(1, 0, 2))


# ------------------------------------------------------------------ host prep
def _prep_in_maps(inp):
    f32 = np.float32
    qT = np.ascontiguousarray(np.transpose(np.asarray(inp["queries"], f32), (0, 2, 1)))
    vT = np.ascontiguousarray(np.transpose(np.asarray(inp["values"], f32), (0, 2, 1)))
    pw = np.asarray(inp["pos_w"], f32)                        # [F, D]
    poswT = np.ascontiguousarray(
        pw.reshape(F, H, DIM).transpose(2, 1, 0)).astype(BF16)  # [d, h, f]

    rp = np.asarray(inp["relative_positions"], f32)
    rpT = np.transpose(rp, (0, 1, 3, 2))                       # [B,S,F,S]
    mask = np.where(np.arange(S)[None, :] <= np.arange(S)[:, None], 0.0, NEG)
    rpT_ext = np.concatenate(
        [rpT, np.broadcast_to(mask[None, :, None, :], (B, S, 1, S))], axis=2
    ).astype(BF16)                                             # [B,S,F+1,S]

    # fold 1/8 score scale into block0/block2 k-columns of w2 (and b2)
    kcol0 = np.zeros(OUTD[0], bool)
    for h in range(H):
        kcol0[h * 192 + 64:h * 192 + 128] = True
    kcol2 = np.zeros(OUTD[2], bool)
    for h in range(H):
        kcol2[h * 128:h * 128 + 64] = True
    kcols = {0: kcol0, 2: kcol2}

    blocks = []
    for i in range(4):
        p = f"b{i}_"
        g = np.asarray(inp[p + "ln_g"], f32)
        be = np.asarray(inp[p + "ln_b"], f32)
        w1 = np.asarray(inp[p + "w1"], f32)
        b1 = np.asarray(inp[p + "b1"], f32)
        w2 = np.asarray(inp[p + "w2"], f32).copy()
        b2 = np.asarray(inp[p + "b2"], f32).copy()
        if i in kcols:
            w2[:, kcols[i]] *= 0.125
            b2[kcols[i]] *= 0.125
        blocks.append((g[:, None] * w1, b1 + be @ w1, w2, b2))

    in_maps = []
    for c in range(8):
        g, s = c // G, c % G
        # rp for this core's 64 q tokens: [8 octet][65 f][8 q][256 k]
        rp_c = rpT_ext[g, s * TLOC:(s + 1) * TLOC].transpose(1, 0, 2)  # [65,64,256]
        rp_c = np.ascontiguousarray(
            rp_c.reshape(F + 1, 8, 8, S).transpose(1, 0, 2, 3))
        m = {
            "xT": _pmajor(qT[g]),
            "vT": _pmajor(vT[g]),
            "xTslice": _pmajor(np.ascontiguousarray(
                qT[g][:, s * TLOC:(s + 1) * TLOC])),
            "pos_wT": poswT,
            "rpT": rp_c,
        }
        for i, (w1f, b1f, w2, b2) in enumerate(blocks):
            w1s = np.ascontiguousarray(w1f[:, s * HIDL:(s + 1) * HIDL])
            m[f"w1_{i}"] = _pmajor(w1s).astype(BF16)
            m[f"w1sum_{i}"] = w1s.sum(axis=0, keepdims=True).astype(BF16)
            m[f"b1_{i}"] = np.ascontiguousarray(
                b1f[s * HIDL:(s + 1) * HIDL].reshape(8, P).T).astype(f32)
            m[f"w2_{i}"] = _pmajor(w2[s * HIDL:(s + 1) * HIDL, :]).astype(BF16)
            m[f"b2_{i}"] = np.ascontiguousarray(
                (0.25 * b2).reshape(-1, P).T).astype(f32)
        in_maps.append(m)
    return in_maps


# --------------------------------------------------------------- device build
def _build_nc():
    nc = bacc_mod.Bacc(num_devices=8)
    io = {}
    io["xT"] = nc.declare_dram_parameter("xT", [P, 8, S], FP32, False)
    io["vT"] = nc.declare_dram_parameter("vT", [P, 8, S], FP32, False)
    io["xTslice"] = nc.declare_dram_parameter("xTslice", [P, 8, TLOC], FP32, False)
    io["pos_wT"] = nc.declare_dram_parameter("pos_wT", [DIM, H, F], BF, False)
    io["rpT"] = nc.declare_dram_parameter("rpT", [8, F + 1, 8, S], BF, False)
    for i in range(4):
        io[f"w1_{i}"] = nc.declare_dram_parameter(f"w1_{i}", [P, 8, HIDL], BF, False)
        io[f"w1sum_{i}"] = nc.declare_dram_parameter(f"w1sum_{i}", [1, HIDL], BF, False)
        io[f"b1_{i}"] = nc.declare_dram_parameter(f"b1_{i}", [P, 8], FP32, False)
        io[f"w2_{i}"] = nc.declare_dram_parameter(f"w2_{i}", [P, 8, OUTD[i]], BF, False)
        io[f"b2_{i}"] = nc.declare_dram_parameter(
            f"b2_{i}", [P, OUTD[i] // P], FP32, False)
    io["x2T_out"] = nc.declare_dram_parameter("x2T_out", [D, TLOC], FP32, True)
    io["o3T_out"] = nc.declare_dram_parameter("o3T_out", [D, S], FP32, True)

    with tile.TileContext(nc) as tc:
        _program(nc, tc, io)
    nc.finalize()
    return nc


def _ap(t, offset, pattern):
    tensor = t.tensor if isinstance(t, bass.AP) else t.ap().tensor
    return bass.AP(tensor=tensor, offset=offset, ap=pattern)


def _program(nc, tc, io):
    import os
    KMODE = int(os.environ.get("KMODE", "9"))
    ctx = contextlib.ExitStack()
    sb = ctx.enter_context(tc.tile_pool(name="sb", bufs=2))
    consts = ctx.enter_context(tc.tile_pool(name="consts", bufs=1))
    psum = ctx.enter_context(tc.tile_pool(name="psum", bufs=2, space="PSUM"))
    dram = ctx.enter_context(tc.tile_pool(name="dram", bufs=1, space="DRAM"))

    sync, vec, act, pe, gps = nc.sync, nc.vector, nc.scalar, nc.tensor, nc.gpsimd
    AF = mybir.ActivationFunctionType
    ALU = mybir.AluOpType

    ones_col = consts.tile([P, 1], BF, tag="ones_col")
    vec.memset(ones_col, 1.0)
    ones_row = consts.tile([1, S], BF, tag="ones_row")
    vec.memset(ones_row, 1.0)

    from concourse.masks import make_identity
    ident = consts.tile([P, P], BF, tag="ident")
    make_identity(nc, ident)

    # ---- t0 prefetches (all contiguous partition-major) ----
    poswT = consts.tile([DIM, H, F], BF, tag="poswT")
    sync.dma_start(out=poswT, in_=io["pos_wT"].ap())
    xs_all = consts.tile([P, 8, TLOC], FP32, tag="xslice")
    sync.dma_start(out=xs_all, in_=io["xTslice"].ap())

    # ---------------------------------------------------------------- helpers
    def ln_scale(xt, T, tag):
        """xt: list of 8 [128,T] f32 SBUF tiles (feature-major).
        Returns (xsc bf16 tiles, negmur bf16 [1,T], xsca full tile)."""
        ps_sum = psum.tile([P, T], FP32, tag="ps_stat", bufs=2)
        ps_sq = psum.tile([P, T], FP32, tag="ps_stat", bufs=2)
        xba = sb.tile([P, 8, T], BF, tag="lnxb", bufs=1)
        sqa = sb.tile([P, 8, T], BF, tag="lnsq", bufs=1)
        for k in range(8):
            act.copy(xba[:, k, :], xt[k])
            pe.matmul(ps_sum[0:1, :], ones_col, xba[:, k, :],
                      start=(k == 0), stop=(k == 7))
        for k in range(8):
            vec.tensor_mul(sqa[:, k, :], xt[k], xt[k])
            pe.matmul(ps_sq[0:1, :], ones_col, sqa[:, k, :],
                      start=(k == 0), stop=(k == 7))
        mu = sb.tile([1, T], FP32, tag="lnstat", bufs=6)
        vec.tensor_scalar_mul(mu, ps_sum[0:1, :], 1.0 / D)
        m2 = sb.tile([1, T], FP32, tag="lnstat", bufs=6)
        vec.tensor_scalar_mul(m2, ps_sq[0:1, :], 1.0 / D)
        ve = sb.tile([1, T], FP32, tag="lnstat", bufs=6)
        vec.tensor_mul(ve, mu, mu)
        vec.tensor_sub(ve, m2, ve)
        vec.tensor_scalar_add(ve, ve, EPS)
        rinv = sb.tile([1, T], FP32, tag="lnstat", bufs=6)
        vec.reciprocal(rinv, ve)
        r = sb.tile([1, T], FP32, tag="lnstat", bufs=6)
        act.sqrt(r, rinv)
        mr = sb.tile([1, T], FP32, tag="lnstat", bufs=6)
        vec.tensor_mul(mr, mu, r)
        negmur = sb.tile([1, T], BF, tag="negmur", bufs=4)
        vec.tensor_scalar_mul(negmur, mr, -1.0)
        rb = sb.tile([1, T], BF, tag="lnstatb", bufs=2)
        vec.tensor_copy(rb, r)
        ps_b = psum.tile([P, T], FP32, tag="ps_stat", bufs=2)
        pe.matmul(ps_b, ones_row[:, 0:P], rb, start=True, stop=True)
        r_bc = sb.tile([P, T], FP32, tag="r_bc", bufs=2)
        vec.tensor_copy(r_bc, ps_b)
        xsca = sb.tile([P, 8, T], BF, tag=f"xsc_{tag}", bufs=1)
        for k in range(8):
            vec.tensor_mul(xsca[:, k, :], xt[k], r_bc)
        return [xsca[:, k, :] for k in range(8)], negmur, xsca

    def load_feature_major(handle, eng=sync):
        xf = sb.tile([P, 8, S], FP32, tag="xf32", bufs=2)
        eng.dma_start(out=xf, in_=handle.ap())
        return [xf[:, k, :] for k in range(8)]

    def ffn(i, mv, negmur, emit, morder=None):
        """mm1 (hidden-split) -> relu -> mm2 row-parallel over local hidden.
        emit(m, ps) consumes the f32 partial [128, S] for out rows m*128.."""
        w1t = sb.tile([P, 8, HIDL], BF, tag="w1", bufs=2)
        act.dma_start(out=w1t, in_=io[f"w1_{i}"].ap())
        w1sum = sb.tile([1, HIDL], BF, tag="w1sum", bufs=2)
        sync.dma_start(out=w1sum, in_=io[f"w1sum_{i}"].ap())
        b1 = consts.tile([P, 8], FP32, tag=f"b1_{i}")
        sync.dma_start(out=b1, in_=io[f"b1_{i}"].ap())
        od = OUTD[i]
        b2 = consts.tile([P, od // P], FP32, tag=f"b2_{i}")
        sync.dma_start(out=b2, in_=io[f"b2_{i}"].ap())
        w2r = sb.tile([P, 8, od], BF, tag="w2", bufs=1,
                      padded_shape=[P, 8, 3 * D])
        act.dma_start(out=w2r, in_=io[f"w2_{i}"].ap())
        h = sb.tile([P, 8, S], BF, tag="h", bufs=2)
        T = mv[0].shape[-1]
        for m in range(8):
            ps = psum.tile([P, S], FP32, tag="ps_mm", bufs=2)
            for k in range(8):
                pe.matmul(ps, w1t[:, k, m * P:(m + 1) * P], mv[k],
                          start=(k == 0), stop=False)
            pe.matmul(ps, w1sum[:, m * P:(m + 1) * P], negmur,
                      start=False, stop=True)
            vec.tensor_scalar(h[:, m, :], ps, b1[:, m:m + 1], 0.0,
                              op0=ALU.add, op1=ALU.max)
        for m in (morder or range(od // P)):
            ps = psum.tile([P, S], FP32, tag="ps_mm", bufs=2)
            for k in range(8):
                pe.matmul(ps, w2r[:, k, m * P:(m + 1) * P], h[:, k, :],
                          start=(k == 0), stop=(k == 7))
            emit(m, b2, ps)

    # =================== LN0 + block0 (fused qkv) ===================
    xt0 = load_feature_major(io["xT"])
    xsc0, negmur0, _ = ln_scale(xt0, S, "ln0")
    # token-reordered partial q -> ReduceScatter; kv partials -> AllReduce
    cc_q0 = dram.tile([G, D, TLOC], BF, tag="cc_q0")
    qg = dram.tile([D, TLOC], BF, tag="qg")
    cc_kv0 = dram.tile([2 * D, S], BF, tag="cc_kv0")
    kvg0 = dram.tile([2 * D, S], BF, tag="kvg0")

    def emit_qkv(m, b2t, ps, cc_q, cc_kv, trip):
        o = sb.tile([P, S], BF, tag="qkvband", bufs=2)
        if m % 2:
            vec.tensor_scalar_add(o, ps, b2t[:, m:m + 1])
        else:
            act.add(o, ps, b2t[:, m:m + 1])
        for j in range(2):
            colbase = m * P + j * 64
            hd, part = colbase // (trip * 64), (colbase % (trip * 64)) // 64
            band = slice(j * 64, (j + 1) * 64)
            if trip == 3 and part == 0:
                sync.dma_start(
                    out=_ap(cc_q, hd * DIM * TLOC,
                            [[TLOC, DIM], [D * TLOC, G], [1, TLOC]]),
                    in_=o[band, :])
            else:
                kvpart = part - 1 if trip == 3 else part
                row = hd * 2 * DIM + kvpart * DIM
                act.dma_start(out=cc_kv[row:row + DIM, :], in_=o[band, :])

    def emit0(m, b2t, ps):
        emit_qkv(m, b2t, ps, cc_q0, cc_kv0, 3)

    # q-containing chunks first so the ReduceScatter can start early
    morder0 = [m for m in range(24) if m % 3 != 2] + [m for m in range(24) if m % 3 == 2]
    ffn(0, xsc0, negmur0, emit0, morder=morder0)
    gps.collective_compute("ReduceScatter", ALU.add, replica_groups=RG,
                           ins=[cc_q0.opt()], outs=[qg.opt()])
    gps.collective_compute("AllReduce", ALU.add, replica_groups=RG,
                           ins=[cc_kv0.opt()], outs=[kvg0.opt()])

    if KMODE < 2:
        ctx.close()
        return

    # =================== LN2 + block2 (cross kv) ===================
    xt2 = load_feature_major(io["vT"], eng=act)
    xsc2, negmur2, _ = ln_scale(xt2, S, "ln2")
    cc_kv2 = dram.tile([2 * D, S], BF, tag="cc_kv2")
    kvg2 = dram.tile([2 * D, S], BF, tag="kvg2")

    def emit2(m, b2t, ps):
        emit_qkv(m, b2t, ps, None, cc_kv2, 2)

    ffn(2, xsc2, negmur2, emit2)
    gps.collective_compute("AllReduce", ALU.add, replica_groups=RG,
                           ins=[cc_kv2.opt()], outs=[kvg2.opt()])

    if KMODE < 3:
        ctx.close()
        return

    # =================== qp + relative-position bias ===================
    # qh: [64(d), 16(h), 64(q)]
    qh_all = sb.tile([DIM, H, TLOC], BF, tag="qh", bufs=1)
    sync.dma_start(out=qh_all, in_=_ap(qg, 0,
                                       [[TLOC, DIM], [DIM * TLOC, H], [1, TLOC]]))
    qp_ext = sb.tile([F + 1, H, TLOC], BF, tag="qp_ext", bufs=1)
    vec.memset(qp_ext[F:F + 1, :, :], 1.0)
    for h in range(H):
        qp_ps = psum.tile([F, TLOC], FP32, tag="ps_attn", bufs=2)
        pe.matmul(qp_ps, poswT[:, h, :], qh_all[:, h, :], start=True, stop=True)
        vec.tensor_copy(qp_ext[0:F, h, :], qp_ps)
    # bias per q-group of 4: bp rows qi*32+h, cols k; batched DRAM round trip.
    # rp streamed per q-octet slab: [65(f), 8(q), 256(k)] contiguous
    bias_d = dram.tile([TLOC // 4, P, S], BF, tag="bias_d")
    for oc in range(TLOC // 8):
        rp8 = sb.tile([F + 1, 8, S], BF, tag="rp8", bufs=1)
        gps.dma_start(out=rp8, in_=_ap(io["rpT"], oc * (F + 1) * 8 * S,
                                       [[8 * S, F + 1], [1, 8 * S]]))
        for g2 in range(2):
            g = oc * 2 + g2
            bp = psum.tile([P, S], FP32, tag="ps_attn", bufs=2)
            for qi in range(4):
                q = g * 4 + qi
                pe.matmul(bp[qi * 32:qi * 32 + H, :], qp_ext[:, :, q],
                          rp8[:, q - oc * 8, :], start=True, stop=True,
                          tile_position=(0, qi * 32))
            bsb = sb.tile([P, S], BF, tag="bsb", bufs=2)
            vec.tensor_copy(bsb, bp)
            act.dma_start(out=bias_d[g], in_=bsb)

    if KMODE < 4:
        ctx.close()
        return

    # =================== attention machinery ===================
    def load_kv(kvg, tag):
        kv_sb = sb.tile([DIM, H, 2, S], BF, tag=tag, bufs=1)
        act.dma_start(out=kv_sb, in_=_ap(kvg, 0,
                                         [[S, DIM], [2 * DIM * S, H],
                                          [DIM * S, 2], [1, S]]))
        return kv_sb

    def make_vT(kv_sb, tag):
        # vT: [128(k-token), 16(h), 2(kc), 64(d)] via PE transposes
        vt = sb.tile([P, H, 2, DIM], BF, tag=tag, bufs=1)
        for h in range(H):
            for kc in range(2):
                tr = psum.tile([P, P], BF, tag="ps_tr", bufs=1)
                pe.matmul(tr[:, 0:DIM], kv_sb[:, h, 1, kc * P:(kc + 1) * P],
                          ident[0:DIM, 0:DIM], is_transpose=True)
                eng = act if (h % 2) else vec
                (eng.copy if eng is act else eng.tensor_copy)(
                    vt[:, h, kc, :], tr[:, 0:DIM])
        return vt

    def softmax_av(s_src, vt, consume):
        """s_src(j) -> f32 scores [128,S] for head pair j (psum or sbuf).
        vt: [128, H, 2, DIM]. consume(j, a_ps) eats psum [128(2h*64d), TLOC]."""
        for j in range(H // 2):
            s_t = s_src(j)
            e = sb.tile([P, S], BF, tag="e", bufs=2)
            act.activation(e, s_t, AF.Exp)
            z = sb.tile([P, 1], FP32, tag="z", bufs=4)
            vec.reduce_sum(z, e, axis=mybir.AxisListType.X)
            rz = sb.tile([P, 1], FP32, tag="rz", bufs=4)
            vec.reciprocal(rz, z)
            en = sb.tile([P, S], BF, tag="en", bufs=2)
            vec.tensor_scalar_mul(en, e, rz)
            wt = sb.tile([P, 2, P], BF, tag="wt", bufs=2)
            for kc in range(2):
                tr = psum.tile([P, P], BF, tag="ps_tr", bufs=1)
                pe.matmul(tr, en[:, kc * P:(kc + 1) * P], ident,
                          is_transpose=True)
                eng = act if kc else vec
                (eng.copy if eng is act else eng.tensor_copy)(wt[:, kc, :], tr)
            a_ps = psum.tile([P, TLOC], FP32, tag="ps_av", bufs=1)
            for kc in range(2):
                for hh in range(2):
                    pe.matmul(a_ps[hh * DIM:(hh + 1) * DIM, :],
                              vt[:, 2 * j + hh, kc, :], wt[:, kc, hh * 64:(hh + 1) * 64],
                              start=(kc == 0), stop=(kc == 1))
            consume(j, a_ps)

    # =================== self-attention ===================
    kv0_sb = load_kv(kvg0, "kv0")
    vt0 = make_vT(kv0_sb, "vt0")

    def s_src0(j):
        s_ps = psum.tile([P, S], FP32, tag="ps_attn", bufs=2)
        for hh in range(2):
            h = 2 * j + hh
            pe.matmul(s_ps[hh * 64:(hh + 1) * 64, :], qh_all[:, h, :],
                      kv0_sb[:, h, 0, :], start=True, stop=True)
        bp = sb.tile([P, S], BF, tag="bp", bufs=2)
        sync.dma_start(
            out=bp, in_=_ap(bias_d.tensor, 2 * j * S,
                            [[S, 2], [P * S, TLOC // 4], [32 * S, 4], [1, S]]))
        vec.tensor_add(s_ps, s_ps, bp)
        return s_ps

    # x1 = queries_slice + a0, built pair by pair
    x1a = sb.tile([P, 8, TLOC], FP32, tag="x1", bufs=1)

    def consume0(j, a_ps):
        vec.tensor_add(x1a[:, j, :], xs_all[:, j, :], a_ps)

    softmax_av(s_src0, vt0, consume0)

    if KMODE < 5:
        ctx.close()
        return

    # LN1 -> AllGather (with negmur row)
    x1t = [x1a[:, k, :] for k in range(8)]
    xsc1, negmur1, xsc1a = ln_scale(x1t, TLOC, "ln1")
    cc_x1 = dram.tile([D + 1, TLOC], BF, tag="cc_x1")
    x1ng = dram.tile([G, D + 1, TLOC], BF, tag="x1ng")
    sync.dma_start(out=_ap(cc_x1, 0, [[TLOC, P], [P * TLOC, 8], [1, TLOC]]),
                   in_=xsc1a)
    sync.dma_start(out=cc_x1[D:D + 1, :], in_=negmur1)
    gps.collective_compute("AllGather", ALU.bypass, replica_groups=RG,
                           ins=[cc_x1.opt()], outs=[x1ng.opt()])

    if KMODE < 6:
        ctx.close()
        return

    # =================== block1 (y = cross-attn queries) ===================
    st1 = (D + 1) * TLOC
    mv1a = sb.tile([P, 8, S], BF, tag="agin", bufs=1)
    for r_ in range(G):
        act.dma_start(out=mv1a[:, :, r_ * TLOC:(r_ + 1) * TLOC],
                      in_=_ap(x1ng.tensor, r_ * st1,
                              [[TLOC, P], [P * TLOC, 8], [1, TLOC]]))
    mv1 = [mv1a[:, k, :] for k in range(8)]
    negmur1f = sb.tile([1, S], BF, tag="negmur", bufs=4)
    sync.dma_start(out=negmur1f, in_=_ap(x1ng.tensor, D * TLOC,
                                         [[1, 1], [st1, G], [1, TLOC]]))
    cc_y = dram.tile([G, D, TLOC], BF, tag="cc_y")
    yg = dram.tile([D, TLOC], BF, tag="yg")

    def emit1(m, b2t, ps):
        o = sb.tile([P, S], BF, tag="qkvband", bufs=2)
        if m % 2:
            vec.tensor_scalar_add(o, ps, b2t[:, m:m + 1])
        else:
            act.add(o, ps, b2t[:, m:m + 1])
        sync.dma_start(
            out=_ap(cc_y, m * P * TLOC, [[TLOC, P], [D * TLOC, G], [1, TLOC]]),
            in_=o)

    ffn(1, mv1, negmur1f, emit1)
    gps.collective_compute("ReduceScatter", ALU.add, replica_groups=RG,
                           ins=[cc_y.opt()], outs=[yg.opt()])

    if KMODE < 7:
        ctx.close()
        return

    # =================== cross-attention ===================
    kv2_sb = load_kv(kvg2, "kv2")
    vt2 = make_vT(kv2_sb, "vt2")
    yh_all = sb.tile([DIM, H, TLOC], BF, tag="yh", bufs=1)
    sync.dma_start(out=yh_all, in_=_ap(yg, 0,
                                       [[TLOC, DIM], [DIM * TLOC, H], [1, TLOC]]))

    def s_src1(j):
        s_ps = psum.tile([P, S], FP32, tag="ps_attn", bufs=2)
        for hh in range(2):
            h = 2 * j + hh
            pe.matmul(s_ps[hh * 64:(hh + 1) * 64, :], yh_all[:, h, :],
                      kv2_sb[:, h, 0, :], start=True, stop=True)
        return s_ps

    # x2 = x1 + a1 -> output + LN3 -> AllGather
    x2a = sb.tile([P, 8, TLOC], FP32, tag="x2", bufs=1)

    def consume1(j, a_ps):
        vec.tensor_add(x2a[:, j, :], x1a[:, j, :], a_ps)

    softmax_av(s_src1, vt2, consume1)

    x2t = [x2a[:, k, :] for k in range(8)]
    sync.dma_start(out=_ap(io["x2T_out"], 0, [[TLOC, P], [P * TLOC, 8], [1, TLOC]]),
                   in_=x2a)
    xsc3, negmur3, xsc3a = ln_scale(x2t, TLOC, "ln3")
    cc_x3 = dram.tile([D + 1, TLOC], BF, tag="cc_x3")
    x3ng = dram.tile([G, D + 1, TLOC], BF, tag="x3ng")
    sync.dma_start(out=_ap(cc_x3, 0, [[TLOC, P], [P * TLOC, 8], [1, TLOC]]),
                   in_=xsc3a)
    sync.dma_start(out=cc_x3[D:D + 1, :], in_=negmur3)
    gps.collective_compute("AllGather", ALU.bypass, replica_groups=RG,
                           ins=[cc_x3.opt()], outs=[x3ng.opt()])

    if KMODE < 8:
        ctx.close()
        return

    # =================== block3 (partials; host sums ranks) ===================
    mv3a = sb.tile([P, 8, S], BF, tag="agin", bufs=1)
    for r_ in range(G):
        act.dma_start(out=mv3a[:, :, r_ * TLOC:(r_ + 1) * TLOC],
                      in_=_ap(x3ng.tensor, r_ * st1,
                              [[TLOC, P], [P * TLOC, 8], [1, TLOC]]))
    mv3 = [mv3a[:, k, :] for k in range(8)]
    negmur3f = sb.tile([1, S], BF, tag="negmur", bufs=4)
    sync.dma_start(out=negmur3f, in_=_ap(x3ng.tensor, D * TLOC,
                                         [[1, 1], [st1, G], [1, TLOC]]))

    def emit3(m, b2t, ps):
        o = sb.tile([P, S], FP32, tag="o3", bufs=1)
        vec.tensor_scalar_add(o, ps, b2t[:, m:m + 1])
        act.dma_start(out=io["o3T_out"].ap()[m * P:(m + 1) * P, :], in_=o)

    ffn(3, mv3, negmur3f, emit3)
    ctx.close()


# ------------------------------------------------------------------- runner
def kernel(**inputs) -> np.ndarray:
    if "nc" not in _CACHE:
        _CACHE["nc"] = _build_nc()
    nc = _CACHE["nc"]
    in_maps = _prep_in_maps(inputs)
    res = run_bass_kernel_spmd(nc, in_maps, core_ids=list(range(8)))
    out = np.zeros((B, S, D), np.float32)
    for g in range(B):
        x2T = np.concatenate(
            [np.asarray(res.results[g * G + s]["x2T_out"]) for s in range(G)], axis=1)
        o3T = np.sum(
            [np.asarray(res.results[g * G + s]["o3T_out"]) for s in range(G)], axis=0)
        out[g] = (x2T + o3T).T
    return out


# revision 17
# speedup vs baseline: 1.4696x; 1.0438x over previous
"""Trainium2 Bass kernel for nn_DecoderWithPositionLayer (8 NeuronCores).

Sharding: 2 groups x 4 cores; group g owns batch g (256 tokens).
Within a group (rank s = core % 4):
  - Every FFN: mm1 hidden-split 4-way (local h = relu(w1_s^T xln + b1_s)),
    mm2 row-parallel over the local hidden slice producing PARTIAL outputs
    for the FULL output dim; partials are combined with a single collective
    AFTER mm2 (no mid-FFN AllGather):
      * block0 q-part / block1 y: token-reordered ReduceScatter(add)
        -> each core gets its 64 query tokens, full feature dim.
      * block0 kv / block2 kv: AllReduce(add) -> full kv on every core.
      * block3: no collective; f32 partials returned, host sums 4 ranks.
  - b1/b2 bias adds ride as rank-1 PE matmuls into the mm PSUM accumulation
    (b2 pre-scaled 0.25 per rank so the collective sum restores it); the
    1/sqrt(dim) score scale is folded into w2/b2 k-columns on the host.
  - Attention split by query tokens (64 q/core). Relative-position bias
    reassociated: bias[q,k] = sum_f qp[q,f] rp[q,k,f],
                  qp[h,q,f] = sum_d q[h,q,d] pos_w[f, h*64+d]
    (pos_b is row-constant in softmax -> dropped). Causal mask rides as an
    extra contraction row of host-transposed rp. Softmax-weight and V
    transposes are PE is_transpose matmuls.
  - All host-side tensors are laid out partition-major so every big DMA
    moves multi-KB contiguous runs per partition.
Activations stay feature-major [feature, token]; matmuls bf16 with f32
PSUM accumulation; LN/softmax math f32.
"""

import contextlib
import numpy as np
import ml_dtypes

import concourse.bass as bass
import concourse.bacc as bacc_mod
import concourse.tile as tile
from concourse import mybir
from concourse.bass_utils import run_bass_kernel_spmd

BF16 = ml_dtypes.bfloat16
FP32 = mybir.dt.float32
BF = mybir.dt.bfloat16

B, S, D, F, HID, H = 2, 256, 1024, 64, 4096, 16
DIM = D // H
G = 4
TLOC = S // G          # 64
HIDL = HID // G        # 1024
P = 128
EPS = 1e-3
NEG = -1e9
OUTD = [3 * D, D, 2 * D, D]
RG = [[0, 1, 2, 3], [4, 5, 6, 7]]

_CACHE = {}


def _pmajor(a):
    """[128*k, N] -> [128, k, N] partition-major contiguous."""
    rows, n = a.shape
    k = rows // P
    return np.ascontiguousarray(a.reshape(k, P, n).transpose(1, 0, 2))


# ------------------------------------------------------------------ host prep
def _prep_in_maps(inp):
    f32 = np.float32
    qT = np.ascontiguousarray(np.transpose(np.asarray(inp["queries"], f32), (0, 2, 1)))
    vT = np.ascontiguousarray(np.transpose(np.asarray(inp["values"], f32), (0, 2, 1)))
    pw = np.asarray(inp["pos_w"], f32)                        # [F, D]
    poswT = np.ascontiguousarray(
        pw.reshape(F, H, DIM).transpose(2, 1, 0)).astype(BF16)  # [d, h, f]

    rp = np.asarray(inp["relative_positions"], f32)
    rpT = np.transpose(rp, (0, 1, 3, 2))                       # [B,S,F,S]
    mask = np.where(np.arange(S)[None, :] <= np.arange(S)[:, None], 0.0, NEG)
    rpT_ext = np.concatenate(
        [rpT, np.broadcast_to(mask[None, :, None, :], (B, S, 1, S))], axis=2
    ).astype(BF16)                                             # [B,S,F+1,S]

    # fold 1/8 score scale into block0/block2 k-columns of w2 (and b2)
    kcol0 = np.zeros(OUTD[0], bool)
    for h in range(H):
        kcol0[h * 192 + 64:h * 192 + 128] = True
    kcol2 = np.zeros(OUTD[2], bool)
    for h in range(H):
        kcol2[h * 128:h * 128 + 64] = True
    kcols = {0: kcol0, 2: kcol2}

    blocks = []
    for i in range(4):
        p = f"b{i}_"
        g = np.asarray(inp[p + "ln_g"], f32)
        be = np.asarray(inp[p + "ln_b"], f32)
        w1 = np.asarray(inp[p + "w1"], f32)
        b1 = np.asarray(inp[p + "b1"], f32)
        w2 = np.asarray(inp[p + "w2"], f32).copy()
        b2 = np.asarray(inp[p + "b2"], f32).copy()
        if i in kcols:
            w2[:, kcols[i]] *= 0.125
            b2[kcols[i]] *= 0.125
        blocks.append((g[:, None] * w1, b1 + be @ w1, w2, b2))

    in_maps = []
    for c in range(8):
        g, s = c // G, c % G
        # rp for this core's 64 q tokens: [8 octet][65 f][8 q][256 k]
        rp_c = rpT_ext[g, s * TLOC:(s + 1) * TLOC].transpose(1, 0, 2)  # [65,64,256]
        rp_c = np.ascontiguousarray(
            rp_c.reshape(F + 1, 8, 8, S).transpose(1, 0, 2, 3))
        m = {
            "xT": _pmajor(qT[g]),
            "vT": _pmajor(vT[g]),
            "xTslice": _pmajor(np.ascontiguousarray(
                qT[g][:, s * TLOC:(s + 1) * TLOC])),
            "pos_wT": poswT,
            "rpT": rp_c,
        }
        for i, (w1f, b1f, w2, b2) in enumerate(blocks):
            w1s = np.ascontiguousarray(w1f[:, s * HIDL:(s + 1) * HIDL])
            m[f"w1_{i}"] = _pmajor(w1s).astype(BF16)
            m[f"w1sum_{i}"] = w1s.sum(axis=0, keepdims=True).astype(BF16)
            m[f"b1_{i}"] = np.ascontiguousarray(
                b1f[s * HIDL:(s + 1) * HIDL].reshape(8, P).T).astype(f32)
            m[f"w2_{i}"] = _pmajor(w2[s * HIDL:(s + 1) * HIDL, :]).astype(BF16)
            m[f"b2_{i}"] = np.ascontiguousarray(
                (0.25 * b2).reshape(-1, P).T).astype(f32)
        in_maps.append(m)
    return in_maps


# --------------------------------------------------------------- device build
def _build_nc():
    nc = bacc_mod.Bacc(num_devices=8)
    io = {}
    io["xT"] = nc.declare_dram_parameter("xT", [P, 8, S], FP32, False)
    io["vT"] = nc.declare_dram_parameter("vT", [P, 8, S], FP32, False)
    io["xTslice"] = nc.declare_dram_parameter("xTslice", [P, 8, TLOC], FP32, False)
    io["pos_wT"] = nc.declare_dram_parameter("pos_wT", [DIM, H, F], BF, False)
    io["rpT"] = nc.declare_dram_parameter("rpT", [8, F + 1, 8, S], BF, False)
    for i in range(4):
        io[f"w1_{i}"] = nc.declare_dram_parameter(f"w1_{i}", [P, 8, HIDL], BF, False)
        io[f"w1sum_{i}"] = nc.declare_dram_parameter(f"w1sum_{i}", [1, HIDL], BF, False)
        io[f"b1_{i}"] = nc.declare_dram_parameter(f"b1_{i}", [P, 8], FP32, False)
        io[f"w2_{i}"] = nc.declare_dram_parameter(f"w2_{i}", [P, 8, OUTD[i]], BF, False)
        io[f"b2_{i}"] = nc.declare_dram_parameter(
            f"b2_{i}", [P, OUTD[i] // P], FP32, False)
    io["x2T_out"] = nc.declare_dram_parameter("x2T_out", [D, TLOC], FP32, True)
    io["o3T_out"] = nc.declare_dram_parameter("o3T_out", [D, S], BF, True)

    with tile.TileContext(nc) as tc:
        _program(nc, tc, io)
    nc.finalize()
    return nc


def _ap(t, offset, pattern):
    tensor = t.tensor if isinstance(t, bass.AP) else t.ap().tensor
    return bass.AP(tensor=tensor, offset=offset, ap=pattern)


def _program(nc, tc, io):
    import os
    KMODE = int(os.environ.get("KMODE", "9"))
    ctx = contextlib.ExitStack()
    sb = ctx.enter_context(tc.tile_pool(name="sb", bufs=2))
    consts = ctx.enter_context(tc.tile_pool(name="consts", bufs=1))
    psum = ctx.enter_context(tc.tile_pool(name="psum", bufs=2, space="PSUM"))
    dram = ctx.enter_context(tc.tile_pool(name="dram", bufs=1, space="DRAM"))

    sync, vec, act, pe, gps = nc.sync, nc.vector, nc.scalar, nc.tensor, nc.gpsimd
    AF = mybir.ActivationFunctionType
    ALU = mybir.AluOpType

    ones_col = consts.tile([P, 1], BF, tag="ones_col")
    vec.memset(ones_col, 1.0)
    ones_row = consts.tile([1, S], BF, tag="ones_row")
    vec.memset(ones_row, 1.0)

    from concourse.masks import make_identity
    ident = consts.tile([P, P], BF, tag="ident")
    make_identity(nc, ident)

    # ---- t0 prefetches (all contiguous partition-major) ----
    poswT = consts.tile([DIM, H, F], BF, tag="poswT")
    sync.dma_start(out=poswT, in_=io["pos_wT"].ap())
    xs_all = consts.tile([P, 8, TLOC], FP32, tag="xslice")
    sync.dma_start(out=xs_all, in_=io["xTslice"].ap())

    # ---------------------------------------------------------------- helpers
    def ln_scale(xt, T, tag):
        """xt: list of 8 [128,T] f32 SBUF tiles (feature-major).
        Returns (xsc bf16 tiles, negmur bf16 [1,T], xsca full tile)."""
        ps_sum = psum.tile([P, T], FP32, tag="ps_stat", bufs=2)
        ps_sq = psum.tile([P, T], FP32, tag="ps_stat", bufs=2)
        xba = sb.tile([P, 8, T], BF, tag="lnxb", bufs=1)
        sqa = sb.tile([P, 8, T], BF, tag="lnsq", bufs=1)
        for k in range(8):
            act.copy(xba[:, k, :], xt[k])
            pe.matmul(ps_sum[0:1, :], ones_col, xba[:, k, :],
                      start=(k == 0), stop=(k == 7))
        for k in range(8):
            vec.tensor_mul(sqa[:, k, :], xt[k], xt[k])
            pe.matmul(ps_sq[0:1, :], ones_col, sqa[:, k, :],
                      start=(k == 0), stop=(k == 7))
        mu = sb.tile([1, T], FP32, tag="lnstat", bufs=6)
        vec.tensor_scalar_mul(mu, ps_sum[0:1, :], 1.0 / D)
        m2 = sb.tile([1, T], FP32, tag="lnstat", bufs=6)
        vec.tensor_scalar_mul(m2, ps_sq[0:1, :], 1.0 / D)
        ve = sb.tile([1, T], FP32, tag="lnstat", bufs=6)
        vec.tensor_mul(ve, mu, mu)
        vec.tensor_sub(ve, m2, ve)
        vec.tensor_scalar_add(ve, ve, EPS)
        rinv = sb.tile([1, T], FP32, tag="lnstat", bufs=6)
        vec.reciprocal(rinv, ve)
        r = sb.tile([1, T], FP32, tag="lnstat", bufs=6)
        act.sqrt(r, rinv)
        mr = sb.tile([1, T], FP32, tag="lnstat", bufs=6)
        vec.tensor_mul(mr, mu, r)
        negmur = sb.tile([1, T], BF, tag="negmur", bufs=4)
        vec.tensor_scalar_mul(negmur, mr, -1.0)
        rb = sb.tile([1, T], BF, tag="lnstatb", bufs=2)
        vec.tensor_copy(rb, r)
        ps_b = psum.tile([P, T], FP32, tag="ps_stat", bufs=2)
        pe.matmul(ps_b, ones_row[:, 0:P], rb, start=True, stop=True)
        r_bc = sb.tile([P, T], FP32, tag="r_bc", bufs=2)
        vec.tensor_copy(r_bc, ps_b)
        xsca = sb.tile([P, 8, T], BF, tag=f"xsc_{tag}", bufs=1)
        for k in range(8):
            vec.tensor_mul(xsca[:, k, :], xt[k], r_bc)
        return [xsca[:, k, :] for k in range(8)], negmur, xsca

    def load_feature_major(handle, eng=sync):
        xf = sb.tile([P, 8, S], FP32, tag="xf32", bufs=2)
        eng.dma_start(out=xf, in_=handle.ap())
        return [xf[:, k, :] for k in range(8)]

    def ffn_weights(i, weng):
        """Prefetch FFN i weights; big tensors on ring `weng`, smalls on act."""
        w1t = w2r = None
        od = OUTD[i]
        if weng is not None:
            w1t = sb.tile([P, 8, HIDL], BF, tag="w1", bufs=2)
            weng.dma_start(out=w1t, in_=io[f"w1_{i}"].ap())
            w2r = sb.tile([P, 8, od], BF, tag="w2", bufs=1,
                          padded_shape=[P, 8, 3 * D])
            weng.dma_start(out=w2r, in_=io[f"w2_{i}"].ap())
        w1sum = sb.tile([1, HIDL], BF, tag="w1sum", bufs=2)
        act.dma_start(out=w1sum, in_=io[f"w1sum_{i}"].ap())
        b1 = consts.tile([P, 8], FP32, tag=f"b1_{i}")
        act.dma_start(out=b1, in_=io[f"b1_{i}"].ap())
        b2 = consts.tile([P, od // P], FP32, tag=f"b2_{i}")
        act.dma_start(out=b2, in_=io[f"b2_{i}"].ap())
        return w1t, w2r, w1sum, b1, b2

    def ffn(i, W, mv, negmur, emit, morder=None):
        """mm1 (hidden-split) -> relu -> mm2 row-parallel over local hidden.
        emit(m, ps) consumes the f32 partial [128, S] for out rows m*128.."""
        w1t, w2r, w1sum, b1, b2 = W
        od = OUTD[i]
        h = sb.tile([P, 8, S], BF, tag="h", bufs=2)
        T = mv[0].shape[-1]
        for m in range(8):
            ps = psum.tile([P, S], FP32, tag="ps_mm", bufs=2)
            for k in range(8):
                pe.matmul(ps, w1t[:, k, m * P:(m + 1) * P], mv[k],
                          start=(k == 0), stop=False)
            pe.matmul(ps, w1sum[:, m * P:(m + 1) * P], negmur,
                      start=False, stop=True)
            vec.tensor_scalar(h[:, m, :], ps, b1[:, m:m + 1], 0.0,
                              op0=ALU.add, op1=ALU.max)
        for m in (morder or range(od // P)):
            ps = psum.tile([P, S], FP32, tag="ps_mm", bufs=2)
            for k in range(8):
                pe.matmul(ps, w2r[:, k, m * P:(m + 1) * P], h[:, k, :],
                          start=(k == 0), stop=(k == 7))
            emit(m, b2, ps)

    # =================== LN0 + block0 (fused qkv) ===================
    # priority load order on the sync ring: xT -> w1_0 -> vT -> w2_0
    xt0 = load_feature_major(io["xT"])
    W0w1 = sb.tile([P, 8, HIDL], BF, tag="w1", bufs=2)
    sync.dma_start(out=W0w1, in_=io["w1_0"].ap())
    xt2 = load_feature_major(io["vT"])
    W0w2 = sb.tile([P, 8, OUTD[0]], BF, tag="w2", bufs=1,
                   padded_shape=[P, 8, 3 * D])
    sync.dma_start(out=W0w2, in_=io["w2_0"].ap())
    W0s = ffn_weights(0, None)
    W0 = (W0w1, W0w2) + W0s[2:]
    W2 = ffn_weights(2, act)
    xsc0, negmur0, _ = ln_scale(xt0, S, "ln0")
    # token-reordered partial q -> ReduceScatter; kv partials -> AllReduce
    cc_q0 = dram.tile([G, D, TLOC], BF, tag="cc_q0")
    qg = dram.tile([D, TLOC], BF, tag="qg")
    cc_kv0 = dram.tile([2 * D, S], BF, tag="cc_kv0")
    kvg0 = dram.tile([2 * D, S], BF, tag="kvg0")

    def emit_qkv(m, b2t, ps, cc_q, cc_kv, trip):
        o = sb.tile([P, S], BF, tag="qkvband", bufs=2)
        if m % 2:
            vec.tensor_scalar_add(o, ps, b2t[:, m:m + 1])
        else:
            act.add(o, ps, b2t[:, m:m + 1])
        for j in range(2):
            colbase = m * P + j * 64
            hd, part = colbase // (trip * 64), (colbase % (trip * 64)) // 64
            band = slice(j * 64, (j + 1) * 64)
            if trip == 3 and part == 0:
                sync.dma_start(
                    out=_ap(cc_q, hd * DIM * TLOC,
                            [[TLOC, DIM], [D * TLOC, G], [1, TLOC]]),
                    in_=o[band, :])
            else:
                kvpart = part - 1 if trip == 3 else part
                row = hd * 2 * DIM + kvpart * DIM
                sync.dma_start(out=cc_kv[row:row + DIM, :], in_=o[band, :])

    def emit0(m, b2t, ps):
        emit_qkv(m, b2t, ps, cc_q0, cc_kv0, 3)

    # q-containing chunks first so the ReduceScatter can start early
    morder0 = [m for m in range(24) if m % 3 != 2] + [m for m in range(24) if m % 3 == 2]
    ffn(0, W0, xsc0, negmur0, emit0, morder=morder0)
    gps.collective_compute("ReduceScatter", ALU.add, replica_groups=RG,
                           ins=[cc_q0.opt()], outs=[qg.opt()])
    gps.collective_compute("AllReduce", ALU.add, replica_groups=RG,
                           ins=[cc_kv0.opt()], outs=[kvg0.opt()])

    if KMODE < 2:
        ctx.close()
        return

    # =================== LN2 + block2 (cross kv) ===================
    xsc2, negmur2, _ = ln_scale(xt2, S, "ln2")
    cc_kv2 = dram.tile([2 * D, S], BF, tag="cc_kv2")
    kvg2 = dram.tile([2 * D, S], BF, tag="kvg2")

    def emit2(m, b2t, ps):
        emit_qkv(m, b2t, ps, None, cc_kv2, 2)

    ffn(2, W2, xsc2, negmur2, emit2)
    gps.collective_compute("AllReduce", ALU.add, replica_groups=RG,
                           ins=[cc_kv2.opt()], outs=[kvg2.opt()])

    if KMODE < 3:
        ctx.close()
        return

    W1 = ffn_weights(1, act)
    W3 = ffn_weights(3, act)

    # =================== qp + relative-position bias ===================
    # qh: [64(d), 16(h), 64(q)]
    qh_all = sb.tile([DIM, H, TLOC], BF, tag="qh", bufs=1)
    sync.dma_start(out=qh_all, in_=_ap(qg, 0,
                                       [[TLOC, DIM], [DIM * TLOC, H], [1, TLOC]]))
    qp_ext = sb.tile([F + 1, H, TLOC], BF, tag="qp_ext", bufs=1)
    vec.memset(qp_ext[F:F + 1, :, :], 1.0)
    for h in range(H):
        qp_ps = psum.tile([F, TLOC], FP32, tag="ps_attn", bufs=2)
        pe.matmul(qp_ps, poswT[:, h, :], qh_all[:, h, :], start=True, stop=True)
        vec.tensor_copy(qp_ext[0:F, h, :], qp_ps)
    # bias per q-group of 4: bp rows qi*32+h, cols k; batched DRAM round trip.
    # rp streamed per q-octet slab: [65(f), 8(q), 256(k)] contiguous
    bias_d = dram.tile([TLOC // 4, P, S], BF, tag="bias_d")
    for oc in range(TLOC // 8):
        rp8 = sb.tile([F + 1, 8, S], BF, tag="rp8", bufs=1)
        gps.dma_start(out=rp8, in_=_ap(io["rpT"], oc * (F + 1) * 8 * S,
                                       [[8 * S, F + 1], [1, 8 * S]]))
        for g2 in range(2):
            g = oc * 2 + g2
            bp = psum.tile([P, S], FP32, tag="ps_attn", bufs=2)
            for qi in range(4):
                q = g * 4 + qi
                pe.matmul(bp[qi * 32:qi * 32 + H, :], qp_ext[:, :, q],
                          rp8[:, q - oc * 8, :], start=True, stop=True,
                          tile_position=(0, qi * 32))
            bsb = sb.tile([P, S], BF, tag="bsb", bufs=2)
            vec.tensor_copy(bsb, bp)
            gps.dma_start(out=bias_d[g], in_=bsb)

    if KMODE < 4:
        ctx.close()
        return

    # =================== attention machinery ===================
    def load_kv(kvg, tag):
        kv_sb = sb.tile([DIM, H, 2, S], BF, tag=tag, bufs=1)
        sync.dma_start(out=kv_sb, in_=_ap(kvg, 0,
                                         [[S, DIM], [2 * DIM * S, H],
                                          [DIM * S, 2], [1, S]]))
        return kv_sb

    def make_vT(kv_sb, tag):
        # vT: [128(k-token), 16(h), 2(kc), 64(d)] via PE transposes
        vt = sb.tile([P, H, 2, DIM], BF, tag=tag, bufs=1)
        for h in range(H):
            for kc in range(2):
                tr = psum.tile([P, P], BF, tag="ps_tr", bufs=2)
                pe.matmul(tr[:, 0:DIM], kv_sb[:, h, 1, kc * P:(kc + 1) * P],
                          ident[0:DIM, 0:DIM], is_transpose=True)
                eng = act if (h % 2) else vec
                (eng.copy if eng is act else eng.tensor_copy)(
                    vt[:, h, kc, :], tr[:, 0:DIM])
        return vt

    def softmax_av(s_src, vt, consume):
        """s_src(j) -> f32 scores [128,S] for head pair j (psum or sbuf).
        vt: [128, H, 2, DIM]. consume(j, a_ps) eats psum [128(2h*64d), TLOC]."""
        for j in range(H // 2):
            s_t = s_src(j)
            e = sb.tile([P, S], BF, tag="e", bufs=2)
            act.activation(e, s_t, AF.Exp)
            z = sb.tile([P, 1], FP32, tag="z", bufs=4)
            vec.reduce_sum(z, e, axis=mybir.AxisListType.X)
            rz = sb.tile([P, 1], FP32, tag="rz", bufs=4)
            vec.reciprocal(rz, z)
            en = sb.tile([P, S], BF, tag="en", bufs=2)
            vec.tensor_scalar_mul(en, e, rz)
            wt = sb.tile([P, 2, P], BF, tag="wt", bufs=2)
            for kc in range(2):
                tr = psum.tile([P, P], BF, tag="ps_tr", bufs=2)
                pe.matmul(tr, en[:, kc * P:(kc + 1) * P], ident,
                          is_transpose=True)
                eng = act if kc else vec
                (eng.copy if eng is act else eng.tensor_copy)(wt[:, kc, :], tr)
            a_ps = psum.tile([P, TLOC], FP32, tag="ps_attn", bufs=2,
                             padded_shape=[P, S])
            for kc in range(2):
                for hh in range(2):
                    pe.matmul(a_ps[hh * DIM:(hh + 1) * DIM, :],
                              vt[:, 2 * j + hh, kc, :], wt[:, kc, hh * 64:(hh + 1) * 64],
                              start=(kc == 0), stop=(kc == 1))
            consume(j, a_ps)

    # =================== self-attention ===================
    kv0_sb = load_kv(kvg0, "kv0")
    vt0 = make_vT(kv0_sb, "vt0")

    def s_src0(j):
        s_ps = psum.tile([P, S], FP32, tag="ps_attn", bufs=2)
        for hh in range(2):
            h = 2 * j + hh
            pe.matmul(s_ps[hh * 64:(hh + 1) * 64, :], qh_all[:, h, :],
                      kv0_sb[:, h, 0, :], start=True, stop=True)
        bp = sb.tile([P, S], BF, tag="bp", bufs=2)
        sync.dma_start(
            out=bp, in_=_ap(bias_d.tensor, 2 * j * S,
                            [[S, 2], [P * S, TLOC // 4], [32 * S, 4], [1, S]]))
        vec.tensor_add(s_ps, s_ps, bp)
        return s_ps

    # x1 = queries_slice + a0, built pair by pair
    x1a = sb.tile([P, 8, TLOC], FP32, tag="x1", bufs=1)

    def consume0(j, a_ps):
        vec.tensor_add(x1a[:, j, :], xs_all[:, j, :], a_ps)

    softmax_av(s_src0, vt0, consume0)

    if KMODE < 5:
        ctx.close()
        return

    # LN1 -> AllGather (with negmur row)
    x1t = [x1a[:, k, :] for k in range(8)]
    xsc1, negmur1, xsc1a = ln_scale(x1t, TLOC, "ln1")
    cc_x1 = dram.tile([D + 1, TLOC], BF, tag="cc_x1")
    x1ng = dram.tile([G, D + 1, TLOC], BF, tag="x1ng")
    sync.dma_start(out=_ap(cc_x1, 0, [[TLOC, P], [P * TLOC, 8], [1, TLOC]]),
                   in_=xsc1a)
    sync.dma_start(out=cc_x1[D:D + 1, :], in_=negmur1)
    gps.collective_compute("AllGather", ALU.bypass, replica_groups=RG,
                           ins=[cc_x1.opt()], outs=[x1ng.opt()])

    if KMODE < 6:
        ctx.close()
        return

    # =================== block1 (y = cross-attn queries) ===================
    st1 = (D + 1) * TLOC
    mv1a = sb.tile([P, 8, S], BF, tag="agin", bufs=1)
    for r_ in range(G):
        sync.dma_start(out=mv1a[:, :, r_ * TLOC:(r_ + 1) * TLOC],
                      in_=_ap(x1ng.tensor, r_ * st1,
                              [[TLOC, P], [P * TLOC, 8], [1, TLOC]]))
    mv1 = [mv1a[:, k, :] for k in range(8)]
    negmur1f = sb.tile([1, S], BF, tag="negmur", bufs=4)
    sync.dma_start(out=negmur1f, in_=_ap(x1ng.tensor, D * TLOC,
                                         [[1, 1], [st1, G], [1, TLOC]]))
    cc_y = dram.tile([G, D, TLOC], BF, tag="cc_y")
    yg = dram.tile([D, TLOC], BF, tag="yg")

    def emit1(m, b2t, ps):
        o = sb.tile([P, S], BF, tag="qkvband", bufs=2)
        if m % 2:
            vec.tensor_scalar_add(o, ps, b2t[:, m:m + 1])
        else:
            act.add(o, ps, b2t[:, m:m + 1])
        sync.dma_start(
            out=_ap(cc_y, m * P * TLOC, [[TLOC, P], [D * TLOC, G], [1, TLOC]]),
            in_=o)

    ffn(1, W1, mv1, negmur1f, emit1)
    gps.collective_compute("ReduceScatter", ALU.add, replica_groups=RG,
                           ins=[cc_y.opt()], outs=[yg.opt()])

    if KMODE < 7:
        ctx.close()
        return

    # =================== cross-attention ===================
    kv2_sb = load_kv(kvg2, "kv2")
    vt2 = make_vT(kv2_sb, "vt2")
    yh_all = sb.tile([DIM, H, TLOC], BF, tag="yh", bufs=1)
    sync.dma_start(out=yh_all, in_=_ap(yg, 0,
                                       [[TLOC, DIM], [DIM * TLOC, H], [1, TLOC]]))

    def s_src1(j):
        s_ps = psum.tile([P, S], FP32, tag="ps_attn", bufs=2)
        for hh in range(2):
            h = 2 * j + hh
            pe.matmul(s_ps[hh * 64:(hh + 1) * 64, :], yh_all[:, h, :],
                      kv2_sb[:, h, 0, :], start=True, stop=True)
        return s_ps

    # x2 = x1 + a1 -> output + LN3 -> AllGather
    x2a = sb.tile([P, 8, TLOC], FP32, tag="x2", bufs=1)

    def consume1(j, a_ps):
        vec.tensor_add(x2a[:, j, :], x1a[:, j, :], a_ps)

    softmax_av(s_src1, vt2, consume1)

    x2t = [x2a[:, k, :] for k in range(8)]
    sync.dma_start(out=_ap(io["x2T_out"], 0, [[TLOC, P], [P * TLOC, 8], [1, TLOC]]),
                   in_=x2a)
    xsc3, negmur3, xsc3a = ln_scale(x2t, TLOC, "ln3")
    cc_x3 = dram.tile([D + 1, TLOC], BF, tag="cc_x3")
    x3ng = dram.tile([G, D + 1, TLOC], BF, tag="x3ng")
    sync.dma_start(out=_ap(cc_x3, 0, [[TLOC, P], [P * TLOC, 8], [1, TLOC]]),
                   in_=xsc3a)
    sync.dma_start(out=cc_x3[D:D + 1, :], in_=negmur3)
    gps.collective_compute("AllGather", ALU.bypass, replica_groups=RG,
                           ins=[cc_x3.opt()], outs=[x3ng.opt()])

    if KMODE < 8:
        ctx.close()
        return

    # =================== block3 (partials; host sums ranks) ===================
    mv3a = sb.tile([P, 8, S], BF, tag="agin", bufs=1)
    for r_ in range(G):
        sync.dma_start(out=mv3a[:, :, r_ * TLOC:(r_ + 1) * TLOC],
                      in_=_ap(x3ng.tensor, r_ * st1,
                              [[TLOC, P], [P * TLOC, 8], [1, TLOC]]))
    mv3 = [mv3a[:, k, :] for k in range(8)]
    negmur3f = sb.tile([1, S], BF, tag="negmur", bufs=4)
    sync.dma_start(out=negmur3f, in_=_ap(x3ng.tensor, D * TLOC,
                                         [[1, 1], [st1, G], [1, TLOC]]))

    def emit3(m, b2t, ps):
        o = sb.tile([P, S], BF, tag="o3", bufs=2)
        vec.tensor_scalar_add(o, ps, b2t[:, m:m + 1])
        act.dma_start(out=io["o3T_out"].ap()[m * P:(m + 1) * P, :], in_=o)

    ffn(3, W3, mv3, negmur3f, emit3)
    ctx.close()


# ------------------------------------------------------------------- runner
def kernel(**inputs) -> np.ndarray:
    if "nc" not in _CACHE:
        _CACHE["nc"] = _build_nc()
    nc = _CACHE["nc"]
    in_maps = _prep_in_maps(inputs)
    res = run_bass_kernel_spmd(nc, in_maps, core_ids=list(range(8)))
    out = np.zeros((B, S, D), np.float32)
    for g in range(B):
        x2T = np.concatenate(
            [np.asarray(res.results[g * G + s]["x2T_out"]) for s in range(G)], axis=1)
        o3T = np.sum(
            [np.asarray(res.results[g * G + s]["o3T_out"]).astype(np.float32)
             for s in range(G)], axis=0)
        out[g] = (x2T + o3T).T
    return out
